# revision 1
# baseline (speedup 1.0000x reference)
"""Trainium2 Bass kernel for nn_CrossAttention.

Sharding: data-parallel over batch (B=8 -> 8 cores, one batch element per
core). No collectives. Host pre-transposes activations/weights into
contraction-on-partition layouts and casts to bf16; all matmuls run at
1 cyc/row on the PE with fp32 PSUM accumulation.

Per-core dataflow (batch b):
  QT  = WqT-matmuls over xT       -> (d', lq)  "transposed" layout
  KT  = WkT-matmuls over ctxT     -> (d', lkv)
  V   = ctxT-matmuls over WvT     -> (lkv, d') + ones column per head
  RMS-norm factors per (head, pos) via PE selector matmuls + Ln/Exp,
     applied through DRAM-bounce partition broadcasts.
  scoresT = khat.T @ qhat per head -> (lkv, lq); exp on ACT with the
     context-mask as per-partition bias; probsT in bf16.
  PV: V (with ones col) @ probsT -> (hd+1, lq): row 64 is the softmax
     denominator. Normalize via Ln/Exp reciprocal + broadcast multiply.
  out = attnT-matmuls over WpT + bias-row matmul -> (lq, d') fp32.
"""

import sys

for _p in ("/opt/trn_rl_repo",):
    if _p not in sys.path:
        sys.path.insert(0, _p)

import numpy as np
import ml_dtypes

import concourse.bass as bass
import concourse.mybir as mybir
import concourse.tile as tile
from concourse import bacc
from concourse import bass_utils

BF16 = mybir.dt.bfloat16
F32 = mybir.dt.float32
BFNP = ml_dtypes.bfloat16

B, LQ, LKV, D, H = 8, 2048, 1024, 1024, 16
HD = D // H          # 64
P = 128              # partitions
DT = D // P          # 8 d-tiles
KT_ = LKV // P       # 8 lkv-tiles
CH = 1024            # lq chunk
NCH = LQ // CH       # 2
NS = CH // 512       # 512-wide matmul slices per chunk
EPS = 1e-6
NEG = -1.0e30

_CACHE = {}
LAST_RESULTS = None


def _patch_act_tables():
    """Restrict usable ACT function sets to natural_log_exp_and_others (it
    contains both Exp and Ln) so the table-load pass never alternates between
    exp_and_others / natural_log — each switch costs ~2.7us on ScalarE.
    Indices (act_func_set_id) are preserved; other sets are just emptied."""
    import concourse.hw_specs as hw_specs
    import concourse.bass_interp as bass_interp

    if getattr(_patch_act_tables, "_done", False):
        return
    orig = hw_specs.get_activation_tables

    def patched(module_arch):
        t = orig(module_arch)
        keep = "natural_log_exp_and_others"
        if keep in t:
            t = {k: (v if k == keep else set()) for k, v in t.items()}
        return t

    hw_specs.get_activation_tables = patched
    bacc.get_activation_tables = patched
    bass_interp.get_activation_tables = patched
    _patch_act_tables._done = True


def _build():
    _patch_act_tables()
    nc = bacc.Bacc("TRN2", target_bir_lowering=False, debug=False)

    xT_d = nc.dram_tensor("xT", (D, LQ), BF16, kind="ExternalInput").ap()
    ctxT_d = nc.dram_tensor("ctxT", (D, LKV), BF16, kind="ExternalInput").ap()
    wqT_d = nc.dram_tensor("wqT", (D, D), BF16, kind="ExternalInput").ap()
    wkT_d = nc.dram_tensor("wkT", (D, D), BF16, kind="ExternalInput").ap()
    wvT_d = nc.dram_tensor("wvT", (D, D), BF16, kind="ExternalInput").ap()
    wpT_d = nc.dram_tensor("wpT", (D, D), BF16, kind="ExternalInput").ap()
    bp_d = nc.dram_tensor("bp", (1, D), BF16, kind="ExternalInput").ap()
    mask_d = nc.dram_tensor("mask", (P, KT_), F32, kind="ExternalInput").ap()
    wqc_d = nc.dram_tensor("wqc", (P, 1), F32, kind="ExternalInput").ap()
    wkc_d = nc.dram_tensor("wkc", (P, 1), F32, kind="ExternalInput").ap()
    out_d = nc.dram_tensor("out", (LQ, D), F32, kind="ExternalOutput").ap()

    with tile.TileContext(nc) as tc:
        _kernel_body(
            nc, tc, xT_d, ctxT_d, wqT_d, wkT_d, wvT_d, wpT_d, bp_d, mask_d,
            wqc_d, wkc_d, out_d,
        )
    nc.compile()
    return nc


def _kernel_body(
    nc, tc, xT_d, ctxT_d, wqT_d, wkT_d, wvT_d, wpT_d, bp_d, mask_d,
    wqc_d, wkc_d, out_d,
):
    import contextlib

    ctx = contextlib.ExitStack()
    with ctx:
        const = ctx.enter_context(tc.tile_pool(name="const", bufs=1))
        mm_ps = ctx.enter_context(tc.tile_pool(name="mm_ps", bufs=2, space="PSUM"))
        sc_ps = ctx.enter_context(tc.tile_pool(name="sc_ps", bufs=2, space="PSUM"))
        pv_ps = ctx.enter_context(tc.tile_pool(name="pv_ps", bufs=1, space="PSUM"))
        dram = ctx.enter_context(tc.tile_pool(name="dram", bufs=2, space="DRAM"))
        work = ctx.enter_context(tc.tile_pool(name="work", bufs=1))
        sq_pool = ctx.enter_context(tc.tile_pool(name="sq", bufs=1))
        rb_pool = ctx.enter_context(tc.tile_pool(name="rb", bufs=2))
        probs_pool = ctx.enter_context(tc.tile_pool(name="probs", bufs=3))
        row_pool = ctx.enter_context(tc.tile_pool(name="row", bufs=2))
        out_pool = ctx.enter_context(tc.tile_pool(name="outp", bufs=2))

        # ---- constants / weights ----
        wq_sb, wk_sb, wv_sb, wp_sb = [], [], [], []
        for k in range(DT):
            for lst, src, nm in (
                (wq_sb, wqT_d, "wq"), (wk_sb, wkT_d, "wk"),
                (wv_sb, wvT_d, "wv"), (wp_sb, wpT_d, "wp"),
            ):
                t = const.tile([P, D], BF16, name=f"{nm}{k}")
                nc.sync.dma_start(t[:], src[P * k : P * (k + 1), :])
                lst.append(t)
        mask_sb = const.tile([P, KT_], F32, name="mask_sb")
        nc.sync.dma_start(mask_sb[:], mask_d[:])
        wqc_sb = const.tile([P, 1], F32, name="wqc_sb")
        nc.sync.dma_start(wqc_sb[:], wqc_d[:])
        wkc_sb = const.tile([P, 1], F32, name="wkc_sb")
        nc.sync.dma_start(wkc_sb[:], wkc_d[:])
        bp_sb = const.tile([1, D], BF16, name="bp_sb")
        nc.sync.dma_start(bp_sb[:], bp_d[:])
        ones_row = const.tile([1, P], BF16, name="ones_row")
        nc.vector.memset(ones_row[:], 1.0)
        eps16 = const.tile([16, 1], F32, name="eps16")
        nc.vector.memset(eps16[:], EPS)
        zero16 = const.tile([16, 1], F32, name="zero16")
        nc.vector.memset(zero16[:], 0.0)
        zero1 = const.tile([1, 1], F32, name="zero1")
        nc.vector.memset(zero1[:], 0.0)
        sel16 = []
        for m in range(DT):
            s = const.tile([P, 16], BF16, name=f"sel{m}")
            nc.vector.memset(s[:], 0.0)
            nc.vector.memset(s[0:64, 2 * m : 2 * m + 1], 1.0)
            nc.vector.memset(s[64:128, 2 * m + 1 : 2 * m + 2], 1.0)
            sel16.append(s)

        khat = [const.tile([P, LKV], BF16, name=f"khat{m}") for m in range(DT)]
        vsb = [const.tile([P, H * (HD + 1)], BF16, name=f"vsb{m}") for m in range(KT_)]
        qhat = [const.tile([P, CH], BF16, name=f"qhat{m}") for m in range(DT)]
        attn = [const.tile([P, CH], BF16, name=f"attn{m}") for m in range(DT)]

        def bcast_rows(rs_sb, ncols, wcol, nm):
            """(16, ncols) bf16 rows -> per-d-tile (128, ncols) bf16 tiles:
            row 2m+j broadcast to partitions 64j..64j+63, times wcol[p]."""
            bounce = dram.tile([16, ncols], BF16, name=f"dr_{nm}", tag=f"dr_{nm}")
            nc.sync.dma_start(bounce[:], rs_sb[:])
            tiles = []
            for m in range(DT):
                rb = rb_pool.tile([P, ncols], BF16, name=f"rb_{nm}{m}", tag="rb")
                for j in range(2):
                    nc.sync.dma_start(
                        rb[64 * j : 64 * (j + 1), :],
                        bounce[2 * m + j : 2 * m + j + 1, :].broadcast_to((64, ncols)),
                    )
                nc.vector.tensor_scalar(
                    rb[:], rb[:], wcol[:], None, mybir.AluOpType.mult
                )
                tiles.append(rb)
            return tiles

        def project(dst_tiles, w_tiles, act_tiles, ncols, scalar_col, nm):
            """dst[m][:, :] (bf16) = (W @ act) for d'-tile m, then RMS-norm
            applied in place via selector-matmul stats + Ln/Exp + broadcast."""
            for m in range(DT):
                for n in range(ncols // 512):
                    ps = mm_ps.tile([P, 512], F32, name=f"ps_{nm}", tag="mm")
                    for k in range(DT):
                        nc.tensor.matmul(
                            ps[:],
                            w_tiles[k][:, P * m : P * (m + 1)],
                            act_tiles[k][:, 512 * n : 512 * (n + 1)],
                            start=(k == 0), stop=(k == DT - 1),
                        )
                    nc.vector.tensor_copy(
                        dst_tiles[m][:, 512 * n : 512 * (n + 1)], ps[:]
                    )
            sq_tiles = []
            for m in range(DT):
                sq = sq_pool.tile([P, ncols], BF16, name=f"sq_{nm}{m}", tag=f"sq{m}")
                nc.vector.tensor_tensor(
                    sq[:], dst_tiles[m][:], dst_tiles[m][:], mybir.AluOpType.mult
                )
                sq_tiles.append(sq)
            ln_t = work.tile([16, ncols], F32, name=f"ln_{nm}", tag="ln")
            for n in range(ncols // 512):
                st = mm_ps.tile([16, 512], F32, name=f"stp_{nm}", tag="mm")
                for m in range(DT):
                    nc.tensor.matmul(
                        st[:], sel16[m][:], sq_tiles[m][:, 512 * n : 512 * (n + 1)],
                        start=(m == 0), stop=(m == DT - 1),
                    )
                nc.scalar.activation(
                    ln_t[:, 512 * n : 512 * (n + 1)], st[:],
                    mybir.ActivationFunctionType.Ln,
                    bias=eps16[:], scale=1.0 / HD,
                )
            rs = work.tile([16, ncols], BF16, name=f"rs_{nm}", tag="rs")
            nc.scalar.activation(
                rs[:], ln_t[:], mybir.ActivationFunctionType.Exp,
                bias=zero16[:], scale=-0.5,
            )
            rbt = bcast_rows(rs, ncols, scalar_col, nm)
            for m in range(DT):
                nc.vector.tensor_tensor(
                    dst_tiles[m][:], dst_tiles[m][:], rbt[m][:], mybir.AluOpType.mult
                )

        # ================= K / V stage =================
        ctx_sb = []
        for k in range(DT):
            t = work.tile([P, LKV], BF16, name=f"ctx{k}", tag=f"io{k}")
            nc.sync.dma_start(t[:], ctxT_d[P * k : P * (k + 1), :])
            ctx_sb.append(t)

        project(khat, wk_sb, ctx_sb, LKV, wkc_sb, "k")

        for m in range(KT_):
            for n in range(2):
                ps = mm_ps.tile([P, 512], F32, name="ps_v", tag="mm")
                for k in range(DT):
                    nc.tensor.matmul(
                        ps[:],
                        ctx_sb[k][:, P * m : P * (m + 1)],
                        wv_sb[k][:, 512 * n : 512 * (n + 1)],
                        start=(k == 0), stop=(k == DT - 1),
                    )
                v3 = vsb[m][:].rearrange("p (h e) -> p h e", e=HD + 1)
                nc.vector.tensor_copy(
                    v3[:, 8 * n : 8 * (n + 1), 0:HD],
                    ps[:].rearrange("p (h e) -> p h e", e=HD),
                )
            v3 = vsb[m][:].rearrange("p (h e) -> p h e", e=HD + 1)
            nc.vector.memset(v3[:, :, HD : HD + 1], 1.0)

        # ================= per-chunk: Q proj + attention + out proj ========
        for c in range(NCH):
            x_sb = []
            for k in range(DT):
                t = work.tile([P, CH], BF16, name=f"x{k}", tag=f"io{k}")
                nc.sync.dma_start(t[:], xT_d[P * k : P * (k + 1), CH * c : CH * (c + 1)])
                x_sb.append(t)

            project(qhat, wq_sb, x_sb, CH, wqc_sb, f"q{c}")

            lnall = work.tile([16, CH], F32, name=f"lnall{c}", tag="lnall")
            for h in range(H):
                mt, off = h // 2, 64 * (h % 2)
                pv = pv_ps.tile([HD + 1, CH], F32, name="pv", tag="pv")
                for t in range(KT_):
                    sc = sc_ps.tile([P, CH], F32, name="sc", tag="sc")
                    for n in range(NS):
                        nc.tensor.matmul(
                            sc[:, 512 * n : 512 * (n + 1)],
                            khat[mt][off : off + HD, P * t : P * (t + 1)],
                            qhat[mt][off : off + HD, 512 * n : 512 * (n + 1)],
                            start=True, stop=True,
                        )
                    pr = probs_pool.tile([P, CH], BF16, name="pr", tag="pr")
                    nc.scalar.activation(
                        pr[:], sc[:], mybir.ActivationFunctionType.Exp,
                        bias=mask_sb[:, t : t + 1], scale=1.0,
                    )
                    for n in range(NS):
                        nc.tensor.matmul(
                            pv[:, 512 * n : 512 * (n + 1)],
                            vsb[t][:, (HD + 1) * h : (HD + 1) * (h + 1)],
                            pr[:, 512 * n : 512 * (n + 1)],
                            start=(t == 0), stop=(t == KT_ - 1),
                        )
                nc.vector.tensor_copy(attn[mt][off : off + HD, :], pv[0:HD, :])
                lnrow = row_pool.tile([1, CH], F32, name="lnrow", tag="lnrow")
                nc.scalar.activation(
                    lnrow[:], pv[HD : HD + 1, :], mybir.ActivationFunctionType.Ln,
                    bias=zero1[:], scale=1.0,
                )
                nc.sync.dma_start(lnall[h : h + 1, :], lnrow[:])

            recip = work.tile([16, CH], BF16, name=f"recip{c}", tag="recip")
            nc.scalar.activation(
                recip[:], lnall[:], mybir.ActivationFunctionType.Exp,
                bias=zero16[:], scale=-1.0,
            )
            rbounce = dram.tile([16, CH], BF16, name=f"rcp{c}", tag="rcp")
            nc.sync.dma_start(rbounce[:], recip[:])
            for m in range(DT):
                rb = rb_pool.tile([P, CH], BF16, name=f"rbn{m}", tag="rb")
                for j in range(2):
                    nc.sync.dma_start(
                        rb[64 * j : 64 * (j + 1), :],
                        rbounce[2 * m + j : 2 * m + j + 1, :].broadcast_to((64, CH)),
                    )
                nc.vector.tensor_tensor(
                    attn[m][:], attn[m][:], rb[:], mybir.AluOpType.mult
                )

            for m in range(CH // P):
                for n in range(2):
                    ps = mm_ps.tile([P, 512], F32, name="ps_o", tag="mm")
                    for k in range(DT):
                        nc.tensor.matmul(
                            ps[:],
                            attn[k][:, P * m : P * (m + 1)],
                            wp_sb[k][:, 512 * n : 512 * (n + 1)],
                            start=(k == 0), stop=False,
                        )
                    nc.tensor.matmul(
                        ps[:], ones_row[:], bp_sb[:, 512 * n : 512 * (n + 1)],
                        start=False, stop=True,
                    )
                    o_sb = out_pool.tile([P, 512], F32, name="o_sb", tag="o")
                    nc.vector.tensor_copy(o_sb[:], ps[:])
                    nc.sync.dma_start(
                        out_d[CH * c + P * m : CH * c + P * (m + 1),
                              512 * n : 512 * (n + 1)],
                        o_sb[:],
                    )


def _prep_inputs(x, context, context_mask, Wq, Wk, Wv, Wp, bp, q_norm_w, k_norm_w):
    scale = HD ** -0.5
    shared = {
        "wqT": np.ascontiguousarray(Wq.T).astype(BFNP),
        "wkT": np.ascontiguousarray(Wk.T).astype(BFNP),
        "wvT": np.ascontiguousarray(Wv.T).astype(BFNP),
        "wpT": np.ascontiguousarray(Wp.T).astype(BFNP),
        "bp": bp.reshape(1, D).astype(BFNP),
        "wqc": np.tile(q_norm_w.astype(np.float64) * scale, 2)
        .reshape(P, 1).astype(np.float32),
        "wkc": np.tile(k_norm_w, 2).reshape(P, 1).astype(np.float32),
    }
    in_maps = []
    for b in range(B):
        m = context_mask[b].astype(bool).copy()
        if not m.any():
            m[0] = True
        bias = np.where(m, 0.0, NEG).astype(np.float32)
        in_maps.append(
            dict(
                shared,
                xT=np.ascontiguousarray(x[b].T).astype(BFNP),
                ctxT=np.ascontiguousarray(context[b].T).astype(BFNP),
                mask=np.ascontiguousarray(bias.reshape(KT_, P).T),
            )
        )
    return in_maps


def kernel(x, context, context_mask, Wq, Wk, Wv, Wp, bp, q_norm_w, k_norm_w):
    global LAST_RESULTS
    x = np.asarray(x, dtype=np.float32)
    context = np.asarray(context, dtype=np.float32)
    context_mask = np.asarray(context_mask)
    if "nc" not in _CACHE:
        _CACHE["nc"] = _build()
    nc = _CACHE["nc"]
    in_maps = _prep_inputs(
        x, context, context_mask,
        np.asarray(Wq, np.float32), np.asarray(Wk, np.float32),
        np.asarray(Wv, np.float32), np.asarray(Wp, np.float32),
        np.asarray(bp, np.float32), np.asarray(q_norm_w, np.float32),
        np.asarray(k_norm_w, np.float32),
    )
    res = bass_utils.run_bass_kernel_spmd(nc, in_maps, core_ids=list(range(B)))
    LAST_RESULTS = res
    return np.stack([res.results[b]["out"] for b in range(B)], axis=0)



# revision 7
# speedup vs baseline: 1.0329x; 1.0329x over previous
"""Trainium2 Bass kernel for nn_CrossAttention.

Sharding: data-parallel over batch (B=8 -> 8 cores, one batch element per
core). No collectives. Host pre-transposes activations/weights into
contraction-on-partition layouts and casts to bf16; all matmuls run at
1 cyc/row on the PE with fp32 PSUM accumulation.

Per-core dataflow (batch b):
  QT  = WqT-matmuls over xT       -> (d', lq)  "transposed" layout
  KT  = WkT-matmuls over ctxT     -> (d', lkv)
  V   = ctxT-matmuls over WvT     -> (lkv, d') + ones column per head
  RMS-norm factors per (head, pos) via PE selector matmuls + Ln/Exp,
     applied through DRAM-bounce partition broadcasts.
  scoresT = khat.T @ qhat per head -> (lkv, lq); exp on ACT with the
     context-mask as per-partition bias; probsT in bf16.
  PV: V (with ones col) @ probsT -> (hd+1, lq): row 64 is the softmax
     denominator. Normalize via DVE reciprocal + broadcast multiply.
  out = attnT-matmuls over WpT + bias-row matmul -> (lq, d') fp32.

Pipelining: qhat/attn are double-buffered so chunk c+1's Q-projection
fills PE bubbles during chunk c's (ACT-bound) attention, and chunk c's
out-projection is emitted interleaved into chunk c+1's attention heads.
Weight SBUF slots are reused (Wk->Wq, Wv->Wp).
"""

import sys

for _p in ("/opt/trn_rl_repo",):
    if _p not in sys.path:
        sys.path.insert(0, _p)

import numpy as np
import ml_dtypes

import concourse.bass as bass
import concourse.mybir as mybir
import concourse.tile as tile
from concourse import bacc
from concourse import bass_utils

BF16 = mybir.dt.bfloat16
F32 = mybir.dt.float32
BFNP = ml_dtypes.bfloat16

B, LQ, LKV, D, H = 8, 2048, 1024, 1024, 16
HD = D // H          # 64
P = 128              # partitions
DT = D // P          # 8 d-tiles
KT_ = LKV // P       # 8 lkv-tiles
CH = 1024            # lq chunk
NCH = LQ // CH       # 2
NS = CH // 512       # 512-wide matmul slices per chunk
EPS = 1e-6
NEG = -1.0e30

_CACHE = {}
LAST_RESULTS = None


def _patch_act_tables():
    """Restrict usable ACT function sets to natural_log_exp_and_others (it
    contains both Exp and Ln) so the table-load pass never alternates between
    exp_and_others / natural_log — each switch costs ~2.7us on ScalarE.
    Indices (act_func_set_id) are preserved; other sets are just emptied."""
    import concourse.hw_specs as hw_specs
    import concourse.bass_interp as bass_interp

    if getattr(_patch_act_tables, "_done", False):
        return
    orig = hw_specs.get_activation_tables

    def patched(module_arch):
        t = orig(module_arch)
        keep = "natural_log_exp_and_others"
        if keep in t:
            t = {k: (v if k == keep else set()) for k, v in t.items()}
        return t

    hw_specs.get_activation_tables = patched
    bacc.get_activation_tables = patched
    bass_interp.get_activation_tables = patched
    _patch_act_tables._done = True


def _build():
    _patch_act_tables()
    nc = bacc.Bacc("TRN2", target_bir_lowering=False, debug=False)

    xT_d = nc.dram_tensor("xT", (D, LQ), BF16, kind="ExternalInput").ap()
    ctxT_d = nc.dram_tensor("ctxT", (D, LKV), BF16, kind="ExternalInput").ap()
    wqT_d = nc.dram_tensor("wqT", (D, D), BF16, kind="ExternalInput").ap()
    wkT_d = nc.dram_tensor("wkT", (D, D), BF16, kind="ExternalInput").ap()
    wvT_d = nc.dram_tensor("wvT", (D, D), BF16, kind="ExternalInput").ap()
    wpT_d = nc.dram_tensor("wpT", (D, D), BF16, kind="ExternalInput").ap()
    bp_d = nc.dram_tensor("bp", (1, D), BF16, kind="ExternalInput").ap()
    mask_d = nc.dram_tensor("mask", (P, KT_), F32, kind="ExternalInput").ap()
    wqc_d = nc.dram_tensor("wqc", (P, 1), F32, kind="ExternalInput").ap()
    wkc_d = nc.dram_tensor("wkc", (P, 1), F32, kind="ExternalInput").ap()
    out_d = nc.dram_tensor("out", (LQ, D), F32, kind="ExternalOutput").ap()

    with tile.TileContext(nc) as tc:
        _kernel_body(
            nc, tc, xT_d, ctxT_d, wqT_d, wkT_d, wvT_d, wpT_d, bp_d, mask_d,
            wqc_d, wkc_d, out_d,
        )
    nc.compile()
    return nc


def _kernel_body(
    nc, tc, xT_d, ctxT_d, wqT_d, wkT_d, wvT_d, wpT_d, bp_d, mask_d,
    wqc_d, wkc_d, out_d,
):
    import contextlib

    ctx = contextlib.ExitStack()
    with ctx:
        const = ctx.enter_context(tc.tile_pool(name="const", bufs=1))
        wpool = ctx.enter_context(tc.tile_pool(name="wpool", bufs=1))
        xio = ctx.enter_context(tc.tile_pool(name="xio", bufs=1))
        dbuf = ctx.enter_context(tc.tile_pool(name="dbuf", bufs=2))
        mm_ps = ctx.enter_context(tc.tile_pool(name="mm_ps", bufs=2, space="PSUM"))
        sc_ps = ctx.enter_context(tc.tile_pool(name="sc_ps", bufs=2, space="PSUM"))
        pv_ps = ctx.enter_context(tc.tile_pool(name="pv_ps", bufs=1, space="PSUM"))
        dram = ctx.enter_context(tc.tile_pool(name="dram", bufs=2, space="DRAM"))
        work = ctx.enter_context(tc.tile_pool(name="work", bufs=1))
        sq_pool = ctx.enter_context(tc.tile_pool(name="sq", bufs=1))
        rb_pool = ctx.enter_context(tc.tile_pool(name="rb", bufs=2))
        probs_pool = ctx.enter_context(tc.tile_pool(name="probs", bufs=2))
        out_pool = ctx.enter_context(tc.tile_pool(name="outp", bufs=2))
        rowp = ctx.enter_context(tc.tile_pool(name="rowp", bufs=1))

        # ---- small constants ----
        mask_sb = const.tile([P, KT_], F32, name="mask_sb")
        nc.sync.dma_start(mask_sb[:], mask_d[:])
        wqc_sb = const.tile([P, 1], F32, name="wqc_sb")
        nc.sync.dma_start(wqc_sb[:], wqc_d[:])
        wkc_sb = const.tile([P, 1], F32, name="wkc_sb")
        nc.sync.dma_start(wkc_sb[:], wkc_d[:])
        bp_sb = const.tile([1, D], BF16, name="bp_sb")
        nc.sync.dma_start(bp_sb[:], bp_d[:])
        ones_row = const.tile([1, P], BF16, name="ones_row")
        nc.vector.memset(ones_row[:], 1.0)
        eps16 = const.tile([16, 1], F32, name="eps16")
        nc.vector.memset(eps16[:], EPS)
        zero16 = const.tile([16, 1], F32, name="zero16")
        nc.vector.memset(zero16[:], 0.0)
        sel16 = []
        for m in range(DT):
            s = const.tile([P, 16], BF16, name=f"sel{m}")
            nc.vector.memset(s[:], 0.0)
            nc.vector.memset(s[0:64, 2 * m : 2 * m + 1], 1.0)
            nc.vector.memset(s[64:128, 2 * m + 1 : 2 * m + 2], 1.0)
            sel16.append(s)

        khat = [const.tile([P, LKV], BF16, name=f"khat{m}") for m in range(DT)]
        vsb = [const.tile([P, H * (HD + 1)], BF16, name=f"vsb{m}") for m in range(KT_)]

        # ---- ctx + stage-A weights (Wk, Wv) ----
        ctx_sb = []
        for k in range(DT):
            t = xio.tile([P, LKV], BF16, name=f"ctx{k}", tag=f"ctx{k}")
            nc.sync.dma_start(t[:], ctxT_d[P * k : P * (k + 1), :])
            ctx_sb.append(t)
        wk_sb, wv_sb = [], []
        for k in range(DT):
            t = wpool.tile([P, D], BF16, name=f"wk{k}", tag=f"wa{k}")
            nc.sync.dma_start(t[:], wkT_d[P * k : P * (k + 1), :])
            wk_sb.append(t)
            t = wpool.tile([P, D], BF16, name=f"wv{k}", tag=f"wb{k}")
            nc.sync.dma_start(t[:], wvT_d[P * k : P * (k + 1), :])
            wv_sb.append(t)

        def bcast_rows(rs_sb, ncols, wcol, nm):
            """(16, ncols) bf16 rows -> per-d-tile (128, ncols) bf16 tiles:
            row 2m+j broadcast to partitions 64j..64j+63, times wcol[p]."""
            bounce = dram.tile([16, ncols], BF16, name=f"dr_{nm}", tag=f"dr_{nm}")
            nc.sync.dma_start(bounce[:], rs_sb[:])
            tiles = []
            for m in range(DT):
                rb = rb_pool.tile([P, ncols], BF16, name=f"rb_{nm}{m}", tag="rb")
                for j in range(2):
                    nc.sync.dma_start(
                        rb[64 * j : 64 * (j + 1), :],
                        bounce[2 * m + j : 2 * m + j + 1, :].broadcast_to((64, ncols)),
                    )
                if wcol is not None:
                    nc.vector.tensor_scalar(
                        rb[:], rb[:], wcol[:], None, mybir.AluOpType.mult
                    )
                tiles.append(rb)
            return tiles

        def project(dst_tiles, w_tiles, act_tiles, ncols, scalar_col, nm):
            """dst[m][:, :] (bf16) = (W @ act) for d'-tile m, then RMS-norm
            applied in place via selector-matmul stats + Ln/Exp + broadcast."""
            for m in range(DT):
                for n in range(ncols // 512):
                    ps = mm_ps.tile([P, 512], F32, name=f"ps_{nm}", tag="mm")
                    for k in range(DT):
                        nc.tensor.matmul(
                            ps[:],
                            w_tiles[k][:, P * m : P * (m + 1)],
                            act_tiles[k][:, 512 * n : 512 * (n + 1)],
                            start=(k == 0), stop=(k == DT - 1),
                        )
                    nc.vector.tensor_copy(
                        dst_tiles[m][:, 512 * n : 512 * (n + 1)], ps[:]
                    )
            sq_tiles = []
            for m in range(DT):
                sq = sq_pool.tile([P, ncols], BF16, name=f"sq_{nm}{m}", tag=f"sq{m}")
                nc.vector.tensor_tensor(
                    sq[:], dst_tiles[m][:], dst_tiles[m][:], mybir.AluOpType.mult
                )
                sq_tiles.append(sq)
            ln_t = work.tile([16, ncols], F32, name=f"ln_{nm}", tag="ln")
            for n in range(ncols // 512):
                st = mm_ps.tile([16, 512], F32, name=f"stp_{nm}", tag="mm")
                for m in range(DT):
                    nc.tensor.matmul(
                        st[:], sel16[m][:], sq_tiles[m][:, 512 * n : 512 * (n + 1)],
                        start=(m == 0), stop=(m == DT - 1),
                    )
                nc.scalar.activation(
                    ln_t[:, 512 * n : 512 * (n + 1)], st[:],
                    mybir.ActivationFunctionType.Ln,
                    bias=eps16[:], scale=1.0 / HD,
                )
            rs = work.tile([16, ncols], BF16, name=f"rs_{nm}", tag="rs")
            nc.scalar.activation(
                rs[:], ln_t[:], mybir.ActivationFunctionType.Exp,
                bias=zero16[:], scale=-0.5,
            )
            rbt = bcast_rows(rs, ncols, scalar_col, nm)
            for m in range(DT):
                nc.vector.tensor_tensor(
                    dst_tiles[m][:], dst_tiles[m][:], rbt[m][:], mybir.AluOpType.mult
                )

        # ================= K / V stage =================
        project(khat, wk_sb, ctx_sb, LKV, wkc_sb, "k")

        for m in range(KT_):
            for n in range(2):
                ps = mm_ps.tile([P, 512], F32, name="ps_v", tag="mm")
                for k in range(DT):
                    nc.tensor.matmul(
                        ps[:],
                        ctx_sb[k][:, P * m : P * (m + 1)],
                        wv_sb[k][:, 512 * n : 512 * (n + 1)],
                        start=(k == 0), stop=(k == DT - 1),
                    )
                v3 = vsb[m][:].rearrange("p (h e) -> p h e", e=HD + 1)
                nc.vector.tensor_copy(
                    v3[:, 8 * n : 8 * (n + 1), 0:HD],
                    ps[:].rearrange("p (h e) -> p h e", e=HD),
                )
            v3 = vsb[m][:].rearrange("p (h e) -> p h e", e=HD + 1)
            nc.vector.memset(v3[:, :, HD : HD + 1], 1.0)

        # ---- stage-B weights (Wq, Wp) reuse the Wk/Wv SBUF slots ----
        wq_sb, wp_sb = [], []
        for k in range(DT):
            t = wpool.tile([P, D], BF16, name=f"wq{k}", tag=f"wa{k}")
            nc.sync.dma_start(t[:], wqT_d[P * k : P * (k + 1), :])
            wq_sb.append(t)
            t = wpool.tile([P, D], BF16, name=f"wp{k}", tag=f"wb{k}")
            nc.sync.dma_start(t[:], wpT_d[P * k : P * (k + 1), :])
            wp_sb.append(t)

        # ================= per-chunk pipeline ==========================
        # chunk c: Q-proj (fills PE during attn(c-1)), attention, normalize;
        # out-proj of chunk c-1 is emitted interleaved into attention(c).
        prev = None  # (attn tiles, chunk idx) pending out-projection

        def emit_outproj(attn_tiles, c, m):
            """out rows [c*CH + m*128, +128) = attnT @ WpT + bias."""
            for n in range(2):
                ps = mm_ps.tile([P, 512], F32, name="ps_o", tag="mm")
                for k in range(DT):
                    nc.tensor.matmul(
                        ps[:],
                        attn_tiles[k][:, P * m : P * (m + 1)],
                        wp_sb[k][:, 512 * n : 512 * (n + 1)],
                        start=(k == 0), stop=False,
                    )
                nc.tensor.matmul(
                    ps[:], ones_row[:], bp_sb[:, 512 * n : 512 * (n + 1)],
                    start=False, stop=True,
                )
                o_sb = out_pool.tile([P, 512], F32, name="o_sb", tag="o")
                nc.vector.tensor_copy(o_sb[:], ps[:])
                nc.sync.dma_start(
                    out_d[CH * c + P * m : CH * c + P * (m + 1),
                          512 * n : 512 * (n + 1)],
                    o_sb[:],
                )

        for c in range(NCH):
            x_sb = []
            for k in range(DT):
                t = xio.tile([P, CH], BF16, name=f"x{k}_{c}", tag=f"x{k}")
                nc.sync.dma_start(t[:], xT_d[P * k : P * (k + 1), CH * c : CH * (c + 1)])
                x_sb.append(t)

            qhat = [dbuf.tile([P, CH], BF16, name=f"qhat{m}_{c}", tag=f"qhat{m}")
                    for m in range(DT)]
            project(qhat, wq_sb, x_sb, CH, wqc_sb, f"q{c}")

            attn = [dbuf.tile([P, CH], BF16, name=f"attn{m}_{c}", tag=f"attn{m}")
                    for m in range(DT)]
            den = work.tile([16, CH], F32, name=f"den{c}", tag="den")

            for h in range(H):
                mt, off = h // 2, 64 * (h % 2)
                pv = pv_ps.tile([HD + 1, CH], F32, name="pv", tag="pv")
                for t in range(KT_):
                    sc = sc_ps.tile([P, CH], F32, name="sc", tag="sc")
                    for n in range(NS):
                        nc.tensor.matmul(
                            sc[:, 512 * n : 512 * (n + 1)],
                            khat[mt][off : off + HD, P * t : P * (t + 1)],
                            qhat[mt][off : off + HD, 512 * n : 512 * (n + 1)],
                            start=True, stop=True,
                        )
                    pr = probs_pool.tile([P, CH], BF16, name="pr", tag="pr")
                    nc.scalar.activation(
                        pr[:], sc[:], mybir.ActivationFunctionType.Exp,
                        bias=mask_sb[:, t : t + 1], scale=1.0,
                    )
                    for n in range(NS):
                        nc.tensor.matmul(
                            pv[:, 512 * n : 512 * (n + 1)],
                            vsb[t][:, (HD + 1) * h : (HD + 1) * (h + 1)],
                            pr[:, 512 * n : 512 * (n + 1)],
                            start=(t == 0), stop=(t == KT_ - 1),
                        )
                nc.vector.tensor_copy(attn[mt][off : off + HD, :], pv[0:HD, :])
                dnr = rowp.tile([1, CH], F32, name="dnr", tag="dnr")
                nc.vector.tensor_copy(dnr[:], pv[HD : HD + 1, :])
                nc.sync.dma_start(den[h : h + 1, :], dnr[:])
                # interleave previous chunk's out-projection into this
                # (ACT-bound) attention window to keep the PE busy:
                # CH/P = 8 q-subtiles spread over 16 heads -> one per even head
                if prev is not None and h % 2 == 0:
                    pattn, pc = prev
                    emit_outproj(pattn, pc, h // 2)
            # ---- normalize: recip(denominator) broadcast multiply ----
            nc.vector.reciprocal(den[:], den[:])
            rcpbf = work.tile([16, CH], BF16, name=f"rcp{c}", tag="rcpbf")
            nc.vector.tensor_copy(rcpbf[:], den[:])
            rbt = bcast_rows(rcpbf, CH, None, f"n{c}")
            for m in range(DT):
                nc.vector.tensor_tensor(
                    attn[m][:], attn[m][:], rbt[m][:], mybir.AluOpType.mult
                )
            prev = (attn, c)

        # final chunk's out-projection (nothing left to overlap with)
        pattn, pc = prev
        for m in range(CH // P):
            emit_outproj(pattn, pc, m)


def _prep_inputs(x, context, context_mask, Wq, Wk, Wv, Wp, bp, q_norm_w, k_norm_w):
    scale = HD ** -0.5
    shared = {
        "wqT": np.ascontiguousarray(Wq.T).astype(BFNP),
        "wkT": np.ascontiguousarray(Wk.T).astype(BFNP),
        "wvT": np.ascontiguousarray(Wv.T).astype(BFNP),
        "wpT": np.ascontiguousarray(Wp.T).astype(BFNP),
        "bp": bp.reshape(1, D).astype(BFNP),
        "wqc": np.tile(q_norm_w.astype(np.float64) * scale, 2)
        .reshape(P, 1).astype(np.float32),
        "wkc": np.tile(k_norm_w, 2).reshape(P, 1).astype(np.float32),
    }
    in_maps = []
    for b in range(B):
        m = context_mask[b].astype(bool).copy()
        if not m.any():
            m[0] = True
        bias = np.where(m, 0.0, NEG).astype(np.float32)
        in_maps.append(
            dict(
                shared,
                xT=np.ascontiguousarray(x[b].T).astype(BFNP),
                ctxT=np.ascontiguousarray(context[b].T).astype(BFNP),
                mask=np.ascontiguousarray(bias.reshape(KT_, P).T),
            )
        )
    return in_maps


def kernel(x, context, context_mask, Wq, Wk, Wv, Wp, bp, q_norm_w, k_norm_w):
    global LAST_RESULTS
    x = np.asarray(x, dtype=np.float32)
    context = np.asarray(context, dtype=np.float32)
    context_mask = np.asarray(context_mask)
    if "nc" not in _CACHE:
        _CACHE["nc"] = _build()
    nc = _CACHE["nc"]
    in_maps = _prep_inputs(
        x, context, context_mask,
        np.asarray(Wq, np.float32), np.asarray(Wk, np.float32),
        np.asarray(Wv, np.float32), np.asarray(Wp, np.float32),
        np.asarray(bp, np.float32), np.asarray(q_norm_w, np.float32),
        np.asarray(k_norm_w, np.float32),
    )
    res = bass_utils.run_bass_kernel_spmd(nc, in_maps, core_ids=list(range(B)))
    LAST_RESULTS = res
    return np.stack([res.results[b]["out"] for b in range(B)], axis=0)


# revision 12
# speedup vs baseline: 1.2546x; 1.2146x over previous
"""Trainium2 Bass kernel for nn_CrossAttention.

Sharding: data-parallel over batch (B=8 -> 8 cores, one batch element per
core). No collectives. Host pre-transposes activations/weights into
contraction-on-partition layouts and casts to bf16; all matmuls run at
1 cyc/row on the PE with fp32 PSUM accumulation.

Per-core dataflow (batch b):
  QT  = WqT-matmuls over xT       -> (d', lq)  "transposed" layout
  KT  = WkT-matmuls over ctxT     -> (d', lkv)
  V   = ctxT-matmuls over WvT     -> (lkv, d') + ones column per head
  RMS-norm factors per (head, pos) via PE selector matmuls + Ln/Exp,
     applied through DRAM-bounce partition broadcasts.
  scoresT = khat.T @ qhat per head -> (lkv, lq); exp on ACT with the
     context-mask as per-partition bias; probsT in bf16.
  PV: V (with ones col) @ probsT -> (hd+1, lq): row 64 is the softmax
     denominator. Normalize via DVE reciprocal + broadcast multiply.
  out = attnT-matmuls over WpT + bias-row matmul -> (lq, d') fp32.

Pipelining: qhat/attn are double-buffered so chunk c+1's Q-projection
fills PE bubbles during chunk c's (ACT-bound) attention, and chunk c's
out-projection is emitted interleaved into chunk c+1's attention heads.
Weight SBUF slots are reused (Wk->Wq, Wv->Wp).
"""

import sys

for _p in ("/opt/trn_rl_repo",):
    if _p not in sys.path:
        sys.path.insert(0, _p)

import numpy as np
import ml_dtypes

import concourse.bass as bass
import concourse.mybir as mybir
import concourse.tile as tile
from concourse import bacc
from concourse import bass_utils

BF16 = mybir.dt.bfloat16
F32 = mybir.dt.float32
BFNP = ml_dtypes.bfloat16

B, LQ, LKV, D, H = 8, 2048, 1024, 1024, 16
HD = D // H          # 64
P = 128              # partitions
DT = D // P          # 8 d-tiles
KT_ = LKV // P       # 8 lkv-tiles
CH = 1024            # lq chunk
NCH = LQ // CH       # 2
NS = CH // 512       # 512-wide matmul slices per chunk
EPS = 1e-6
NEG = -1.0e30

_CACHE = {}
LAST_RESULTS = None


def _patch_act_tables():
    """Restrict usable ACT function sets to natural_log_exp_and_others (it
    contains both Exp and Ln) so the table-load pass never alternates between
    exp_and_others / natural_log — each switch costs ~2.7us on ScalarE.
    Indices (act_func_set_id) are preserved; other sets are just emptied."""
    import concourse.hw_specs as hw_specs
    import concourse.bass_interp as bass_interp

    if getattr(_patch_act_tables, "_done", False):
        return
    orig = hw_specs.get_activation_tables

    def patched(module_arch):
        t = orig(module_arch)
        keep = "natural_log_exp_and_others"
        if keep in t:
            t = {k: (v if k == keep else set()) for k, v in t.items()}
        return t

    hw_specs.get_activation_tables = patched
    bacc.get_activation_tables = patched
    bass_interp.get_activation_tables = patched
    _patch_act_tables._done = True


def _build():
    _patch_act_tables()
    nc = bacc.Bacc("TRN2", target_bir_lowering=False, debug=False)

    xT_d = nc.dram_tensor("xT", (D, LQ), BF16, kind="ExternalInput").ap()
    ctxT_d = nc.dram_tensor("ctxT", (D, LKV), BF16, kind="ExternalInput").ap()
    wqT_d = nc.dram_tensor("wqT", (D, D), BF16, kind="ExternalInput").ap()
    wkT_d = nc.dram_tensor("wkT", (D, D), BF16, kind="ExternalInput").ap()
    wvT_d = nc.dram_tensor("wvT", (D, D), BF16, kind="ExternalInput").ap()
    wpT_d = nc.dram_tensor("wpT", (D, D), BF16, kind="ExternalInput").ap()
    bp_d = nc.dram_tensor("bp", (1, D), BF16, kind="ExternalInput").ap()
    mask_d = nc.dram_tensor("mask", (P, KT_), F32, kind="ExternalInput").ap()
    wqc_d = nc.dram_tensor("wqc", (P, 1), F32, kind="ExternalInput").ap()
    wkc_d = nc.dram_tensor("wkc", (P, 1), F32, kind="ExternalInput").ap()
    out_d = nc.dram_tensor("out", (LQ, D), F32, kind="ExternalOutput").ap()

    with tile.TileContext(nc) as tc:
        _kernel_body(
            nc, tc, xT_d, ctxT_d, wqT_d, wkT_d, wvT_d, wpT_d, bp_d, mask_d,
            wqc_d, wkc_d, out_d,
        )
    nc.compile()
    return nc


def _kernel_body(
    nc, tc, xT_d, ctxT_d, wqT_d, wkT_d, wvT_d, wpT_d, bp_d, mask_d,
    wqc_d, wkc_d, out_d,
):
    import contextlib

    ctx = contextlib.ExitStack()
    with ctx:
        const = ctx.enter_context(tc.tile_pool(name="const", bufs=1))
        wpool = ctx.enter_context(tc.tile_pool(name="wpool", bufs=1))
        xio = ctx.enter_context(tc.tile_pool(name="xio", bufs=1))
        dbuf = ctx.enter_context(tc.tile_pool(name="dbuf", bufs=2))
        mm_ps = ctx.enter_context(tc.tile_pool(name="mm_ps", bufs=2, space="PSUM"))
        sc_ps = ctx.enter_context(tc.tile_pool(name="sc_ps", bufs=2, space="PSUM"))
        pv_ps = ctx.enter_context(tc.tile_pool(name="pv_ps", bufs=1, space="PSUM"))
        dram = ctx.enter_context(tc.tile_pool(name="dram", bufs=2, space="DRAM"))
        work = ctx.enter_context(tc.tile_pool(name="work", bufs=1))
        sq_pool = ctx.enter_context(tc.tile_pool(name="sq", bufs=1))
        # separate slot groups for the q/k-norm broadcasts vs the softmax
        # normalize broadcasts: a shared group serializes chunk c+1's
        # q-norm behind chunk c's normalize, breaking the pipeline
        rbq_pool = ctx.enter_context(tc.tile_pool(name="rbq", bufs=2))
        rbn_pool = ctx.enter_context(tc.tile_pool(name="rbn", bufs=2))
        probs_pool = ctx.enter_context(tc.tile_pool(name="probs", bufs=3))
        out_pool = ctx.enter_context(tc.tile_pool(name="outp", bufs=2))
        rowp = ctx.enter_context(tc.tile_pool(name="rowp", bufs=1))
        ab_pool = ctx.enter_context(tc.tile_pool(name="ab", bufs=1))

        # ---- small constants ----
        mask_sb = const.tile([P, KT_], F32, name="mask_sb")
        nc.sync.dma_start(mask_sb[:], mask_d[:])
        wqc_sb = const.tile([P, 1], F32, name="wqc_sb")
        nc.sync.dma_start(wqc_sb[:], wqc_d[:])
        wkc_sb = const.tile([P, 1], F32, name="wkc_sb")
        nc.sync.dma_start(wkc_sb[:], wkc_d[:])
        bp_sb = const.tile([1, D], BF16, name="bp_sb")
        nc.sync.dma_start(bp_sb[:], bp_d[:])
        ones_row = const.tile([1, P], BF16, name="ones_row")
        nc.vector.memset(ones_row[:], 1.0)
        eps16 = const.tile([16, 1], F32, name="eps16")
        nc.vector.memset(eps16[:], EPS)
        zero16 = const.tile([16, 1], F32, name="zero16")
        nc.vector.memset(zero16[:], 0.0)
        sel16 = []
        for m in range(DT):
            s = const.tile([P, 16], BF16, name=f"sel{m}")
            nc.vector.memset(s[:], 0.0)
            nc.vector.memset(s[0:64, 2 * m : 2 * m + 1], 1.0)
            nc.vector.memset(s[64:128, 2 * m + 1 : 2 * m + 2], 1.0)
            sel16.append(s)

        khat = [const.tile([P, LKV], BF16, name=f"khat{m}") for m in range(DT)]
        vsb = [const.tile([P, H * (HD + 1)], BF16, name=f"vsb{m}") for m in range(KT_)]

        # ---- ctx + stage-A weights (Wk, Wv) ----
        ctx_sb = []
        for k in range(DT):
            t = xio.tile([P, LKV], BF16, name=f"ctx{k}", tag=f"ctx{k}")
            nc.sync.dma_start(t[:], ctxT_d[P * k : P * (k + 1), :])
            ctx_sb.append(t)
        wk_sb, wv_sb = [], []
        for k in range(DT):
            t = wpool.tile([P, D], BF16, name=f"wk{k}", tag=f"wa{k}")
            nc.sync.dma_start(t[:], wkT_d[P * k : P * (k + 1), :])
            wk_sb.append(t)
            t = wpool.tile([P, D], BF16, name=f"wv{k}", tag=f"wb{k}")
            nc.sync.dma_start(t[:], wvT_d[P * k : P * (k + 1), :])
            wv_sb.append(t)

        def bcast_rows(rs_sb, ncols, wcol, nm, pool, tag):
            """(16, ncols) bf16 rows -> per-d-tile (128, ncols) bf16 tiles:
            row 2m+j broadcast to partitions 64j..64j+63, times wcol[p]."""
            bounce = dram.tile([16, ncols], BF16, name=f"dr_{nm}", tag=f"dr_{nm}")
            nc.sync.dma_start(bounce[:], rs_sb[:])
            tiles = []
            for m in range(DT):
                rb = pool.tile([P, ncols], BF16, name=f"rb_{nm}{m}", tag=tag)
                for j in range(2):
                    nc.sync.dma_start(
                        rb[64 * j : 64 * (j + 1), :],
                        bounce[2 * m + j : 2 * m + j + 1, :].broadcast_to((64, ncols)),
                    )
                if wcol is not None:
                    nc.vector.tensor_scalar(
                        rb[:], rb[:], wcol[:], None, mybir.AluOpType.mult
                    )
                tiles.append(rb)
            return tiles

        def project(dst_tiles, w_tiles, act_tiles, ncols, scalar_col, nm):
            """dst[m][:, :] (bf16) = (W @ act) for d'-tile m, then RMS-norm
            applied in place via selector-matmul stats + Ln/Exp + broadcast."""
            for m in range(DT):
                for n in range(ncols // 512):
                    ps = mm_ps.tile([P, 512], F32, name=f"ps_{nm}", tag="mm")
                    for k in range(DT):
                        nc.tensor.matmul(
                            ps[:],
                            w_tiles[k][:, P * m : P * (m + 1)],
                            act_tiles[k][:, 512 * n : 512 * (n + 1)],
                            start=(k == 0), stop=(k == DT - 1),
                        )
                    nc.vector.tensor_copy(
                        dst_tiles[m][:, 512 * n : 512 * (n + 1)], ps[:]
                    )
            sq_tiles = []
            for m in range(DT):
                sq = sq_pool.tile([P, ncols], BF16, name=f"sq_{nm}{m}", tag=f"sq{m}")
                nc.vector.tensor_tensor(
                    sq[:], dst_tiles[m][:], dst_tiles[m][:], mybir.AluOpType.mult
                )
                sq_tiles.append(sq)
            ln_t = work.tile([16, ncols], F32, name=f"ln_{nm}", tag="ln")
            for n in range(ncols // 512):
                st = mm_ps.tile([16, 512], F32, name=f"stp_{nm}", tag="mm")
                for m in range(DT):
                    nc.tensor.matmul(
                        st[:], sel16[m][:], sq_tiles[m][:, 512 * n : 512 * (n + 1)],
                        start=(m == 0), stop=(m == DT - 1),
                    )
                nc.scalar.activation(
                    ln_t[:, 512 * n : 512 * (n + 1)], st[:],
                    mybir.ActivationFunctionType.Ln,
                    bias=eps16[:], scale=1.0 / HD,
                )
            rs = work.tile([16, ncols], BF16, name=f"rs_{nm}", tag="rs")
            nc.scalar.activation(
                rs[:], ln_t[:], mybir.ActivationFunctionType.Exp,
                bias=zero16[:], scale=-0.5,
            )
            rbt = bcast_rows(rs, ncols, scalar_col, nm, rbq_pool, "rbq")
            for m in range(DT):
                nc.vector.tensor_tensor(
                    dst_tiles[m][:], dst_tiles[m][:], rbt[m][:], mybir.AluOpType.mult
                )

        # ================= K / V stage =================
        project(khat, wk_sb, ctx_sb, LKV, wkc_sb, "k")

        for m in range(KT_):
            for n in range(2):
                ps = mm_ps.tile([P, 512], F32, name="ps_v", tag="mm")
                for k in range(DT):
                    nc.tensor.matmul(
                        ps[:],
                        ctx_sb[k][:, P * m : P * (m + 1)],
                        wv_sb[k][:, 512 * n : 512 * (n + 1)],
                        start=(k == 0), stop=(k == DT - 1),
                    )
                v3 = vsb[m][:].rearrange("p (h e) -> p h e", e=HD + 1)
                nc.vector.tensor_copy(
                    v3[:, 8 * n : 8 * (n + 1), 0:HD],
                    ps[:].rearrange("p (h e) -> p h e", e=HD),
                )
            v3 = vsb[m][:].rearrange("p (h e) -> p h e", e=HD + 1)
            nc.vector.memset(v3[:, :, HD : HD + 1], 1.0)

        # ---- stage-B weights (Wq, Wp) reuse the Wk/Wv SBUF slots ----
        wq_sb, wp_sb = [], []
        for k in range(DT):
            t = wpool.tile([P, D], BF16, name=f"wq{k}", tag=f"wa{k}")
            nc.sync.dma_start(t[:], wqT_d[P * k : P * (k + 1), :])
            wq_sb.append(t)
            t = wpool.tile([P, D], BF16, name=f"wp{k}", tag=f"wb{k}")
            nc.sync.dma_start(t[:], wpT_d[P * k : P * (k + 1), :])
            wp_sb.append(t)

        # ================= per-chunk pipeline ==========================
        # chunk c: Q-proj (fills PE during attn(c-1)), attention, normalize;
        # out-proj of chunk c-1 is emitted interleaved into attention(c).
        prev = None  # (attn tiles, chunk idx) pending out-projection

        def emit_outproj(attn_tiles, c, m):
            """out rows [c*CH + m*128, +128) = attnT @ WpT + bias."""
            for n in range(2):
                ps = mm_ps.tile([P, 512], F32, name="ps_o", tag="mm")
                for k in range(DT):
                    nc.tensor.matmul(
                        ps[:],
                        attn_tiles[k][:, P * m : P * (m + 1)],
                        wp_sb[k][:, 512 * n : 512 * (n + 1)],
                        start=(k == 0), stop=False,
                    )
                nc.tensor.matmul(
                    ps[:], ones_row[:], bp_sb[:, 512 * n : 512 * (n + 1)],
                    start=False, stop=True,
                )
                o_sb = out_pool.tile([P, 512], F32, name="o_sb", tag="o")
                nc.vector.tensor_copy(o_sb[:], ps[:])
                nc.sync.dma_start(
                    out_d[CH * c + P * m : CH * c + P * (m + 1),
                          512 * n : 512 * (n + 1)],
                    o_sb[:],
                )

        for c in range(NCH):
            x_sb = []
            for k in range(DT):
                t = xio.tile([P, CH], BF16, name=f"x{k}_{c}", tag=f"x{k}")
                nc.sync.dma_start(t[:], xT_d[P * k : P * (k + 1), CH * c : CH * (c + 1)])
                x_sb.append(t)

            qhat = [dbuf.tile([P, CH], BF16, name=f"qhat{m}_{c}", tag=f"qhat{m}")
                    for m in range(DT)]
            project(qhat, wq_sb, x_sb, CH, wqc_sb, f"q{c}")

            # chunk 0's attn tiles reuse the (dead after V-proj) ctx slots
            if c == 0:
                attn = [xio.tile([P, CH], BF16, name=f"attn{m}_0", tag=f"ctx{m}")
                        for m in range(DT)]
            else:
                attn = [ab_pool.tile([P, CH], BF16, name=f"attn{m}_{c}",
                                     tag=f"attn{m}")
                        for m in range(DT)]
            den = work.tile([16, CH], F32, name=f"den{c}", tag=f"den{c % 2}")

            for h in range(H):
                mt, off = h // 2, 64 * (h % 2)
                pv = pv_ps.tile([HD + 1, CH], F32, name="pv", tag="pv")
                for t in range(KT_):
                    sc = sc_ps.tile([P, CH], F32, name="sc", tag="sc")
                    for n in range(NS):
                        nc.tensor.matmul(
                            sc[:, 512 * n : 512 * (n + 1)],
                            khat[mt][off : off + HD, P * t : P * (t + 1)],
                            qhat[mt][off : off + HD, 512 * n : 512 * (n + 1)],
                            start=True, stop=True,
                        )
                    pr = probs_pool.tile([P, CH], BF16, name="pr", tag="pr")
                    nc.scalar.activation(
                        pr[:], sc[:], mybir.ActivationFunctionType.Exp,
                        bias=mask_sb[:, t : t + 1], scale=1.0,
                    )
                    for n in range(NS):
                        nc.tensor.matmul(
                            pv[:, 512 * n : 512 * (n + 1)],
                            vsb[t][:, (HD + 1) * h : (HD + 1) * (h + 1)],
                            pr[:, 512 * n : 512 * (n + 1)],
                            start=(t == 0), stop=(t == KT_ - 1),
                        )
                nc.vector.tensor_copy(attn[mt][off : off + HD, :], pv[0:HD, :])
                dnr = rowp.tile([1, CH], F32, name="dnr", tag="dnr")
                nc.vector.tensor_copy(dnr[:], pv[HD : HD + 1, :])
                nc.sync.dma_start(den[h : h + 1, :], dnr[:])
                # interleave previous chunk's out-projection into this
                # (ACT-bound) attention window to keep the PE busy:
                # CH/P = 8 q-subtiles spread over 16 heads -> one per even head
                if prev is not None and h % 2 == 0:
                    pattn, pc = prev
                    emit_outproj(pattn, pc, h // 2)
            # ---- normalize: recip(denominator) broadcast multiply ----
            nc.vector.reciprocal(den[:], den[:])
            rcpbf = work.tile([16, CH], BF16, name=f"rcp{c}", tag="rcpbf")
            nc.vector.tensor_copy(rcpbf[:], den[:])
            rbt = bcast_rows(rcpbf, CH, None, f"n{c}", rbn_pool, "rbn")
            for m in range(DT):
                nc.vector.tensor_tensor(
                    attn[m][:], attn[m][:], rbt[m][:], mybir.AluOpType.mult
                )
            prev = (attn, c)

        # final chunk's out-projection (nothing left to overlap with)
        pattn, pc = prev
        for m in range(CH // P):
            emit_outproj(pattn, pc, m)


def _prep_inputs(x, context, context_mask, Wq, Wk, Wv, Wp, bp, q_norm_w, k_norm_w):
    scale = HD ** -0.5
    shared = {
        "wqT": np.ascontiguousarray(Wq.T).astype(BFNP),
        "wkT": np.ascontiguousarray(Wk.T).astype(BFNP),
        "wvT": np.ascontiguousarray(Wv.T).astype(BFNP),
        "wpT": np.ascontiguousarray(Wp.T).astype(BFNP),
        "bp": bp.reshape(1, D).astype(BFNP),
        "wqc": np.tile(q_norm_w.astype(np.float64) * scale, 2)
        .reshape(P, 1).astype(np.float32),
        "wkc": np.tile(k_norm_w, 2).reshape(P, 1).astype(np.float32),
    }
    in_maps = []
    for b in range(B):
        m = context_mask[b].astype(bool).copy()
        if not m.any():
            m[0] = True
        bias = np.where(m, 0.0, NEG).astype(np.float32)
        in_maps.append(
            dict(
                shared,
                xT=np.ascontiguousarray(x[b].T).astype(BFNP),
                ctxT=np.ascontiguousarray(context[b].T).astype(BFNP),
                mask=np.ascontiguousarray(bias.reshape(KT_, P).T),
            )
        )
    return in_maps


def kernel(x, context, context_mask, Wq, Wk, Wv, Wp, bp, q_norm_w, k_norm_w):
    global LAST_RESULTS
    x = np.asarray(x, dtype=np.float32)
    context = np.asarray(context, dtype=np.float32)
    context_mask = np.asarray(context_mask)
    if "nc" not in _CACHE:
        _CACHE["nc"] = _build()
    nc = _CACHE["nc"]
    in_maps = _prep_inputs(
        x, context, context_mask,
        np.asarray(Wq, np.float32), np.asarray(Wk, np.float32),
        np.asarray(Wv, np.float32), np.asarray(Wp, np.float32),
        np.asarray(bp, np.float32), np.asarray(q_norm_w, np.float32),
        np.asarray(k_norm_w, np.float32),
    )
    res = bass_utils.run_bass_kernel_spmd(nc, in_maps, core_ids=list(range(B)))
    LAST_RESULTS = res
    return np.stack([res.results[b]["out"] for b in range(B)], axis=0)


# revision 23
# speedup vs baseline: 1.5669x; 1.2490x over previous
"""Trainium2 Bass kernel for nn_CrossAttention.

Sharding: data-parallel over batch (B=8 -> 8 cores, one batch element per
core). No collectives. Host pre-transposes activations/weights into
contraction-on-partition layouts and casts to bf16; all matmuls run at
1 cyc/row on the PE with fp32 PSUM accumulation.

Per-core dataflow (batch b):
  QT  = WqT-matmuls over xT       -> (d', lq)  "transposed" layout
  KT  = WkT-matmuls over ctxT     -> (d', lkv)
  V   = ctxT-matmuls over WvT     -> (lkv, d') + ones column per head
  RMS-norm factors per (head, pos) via PE selector matmuls + Ln/Exp,
     applied through DRAM-bounce partition broadcasts.
  scoresT = khat.T @ qhat per head -> (lkv, lq); exp on ACT with the
     context-mask as per-partition bias; probsT in bf16.
  PV: V (with ones col) @ probsT -> (hd+1, lq): row 64 is the softmax
     denominator. Normalize via DVE reciprocal + broadcast multiply.
  out = attnT-matmuls over WpT + bias-row matmul -> (lq, d') fp32.

Pipelining: qhat/attn are double-buffered so chunk c+1's Q-projection
fills PE bubbles during chunk c's (ACT-bound) attention, and chunk c's
out-projection is emitted interleaved into chunk c+1's attention heads.
Weight SBUF slots are reused (Wk->Wq, Wv->Wp).
"""

import sys

for _p in ("/opt/trn_rl_repo",):
    if _p not in sys.path:
        sys.path.insert(0, _p)

import numpy as np
import ml_dtypes

import concourse.bass as bass
import concourse.mybir as mybir
import concourse.tile as tile
from concourse import bacc
from concourse import bass_utils

BF16 = mybir.dt.bfloat16
F32 = mybir.dt.float32
BFNP = ml_dtypes.bfloat16

B, LQ, LKV, D, H = 8, 2048, 1024, 1024, 16
HD = D // H          # 64
P = 128              # partitions
DT = D // P          # 8 d-tiles
KT_ = LKV // P       # 8 lkv-tiles
CH = 1024            # lq chunk
NCH = LQ // CH       # 2
NS = CH // 512       # 512-wide matmul slices per chunk
EPS = 1e-6
NEG = -1.0e30

_CACHE = {}
LAST_RESULTS = None


def _patch_act_tables():
    """Restrict usable ACT function sets to natural_log_exp_and_others (it
    contains both Exp and Ln) so the table-load pass never alternates between
    exp_and_others / natural_log — each switch costs ~2.7us on ScalarE.
    Indices (act_func_set_id) are preserved; other sets are just emptied."""
    import concourse.hw_specs as hw_specs
    import concourse.bass_interp as bass_interp

    if getattr(_patch_act_tables, "_done", False):
        return
    orig = hw_specs.get_activation_tables

    def patched(module_arch):
        t = orig(module_arch)
        keep = "natural_log_exp_and_others"
        if keep in t:
            t = {k: (v if k == keep else set()) for k, v in t.items()}
        return t

    hw_specs.get_activation_tables = patched
    bacc.get_activation_tables = patched
    bass_interp.get_activation_tables = patched
    _patch_act_tables._done = True


def _slices(ncols):
    """(offset, width) pairs covering ncols in <=512-wide matmul slices."""
    out, o = [], 0
    while o < ncols:
        w = min(512, ncols - o)
        out.append((o, w))
        o += w
    return out


def _build(lkv):
    """Compile for a compacted context length lkv (multiple of 128)."""
    _patch_act_tables()
    nc = bacc.Bacc("TRN2", target_bir_lowering=False, debug=False)

    kt = lkv // P
    xT_d = nc.dram_tensor("xT", (D, LQ), BF16, kind="ExternalInput").ap()
    ctxT_d = nc.dram_tensor("ctxT", (D, lkv), BF16, kind="ExternalInput").ap()
    wqT_d = nc.dram_tensor("wqT", (D, D), BF16, kind="ExternalInput").ap()
    wkT_d = nc.dram_tensor("wkT", (D, D), BF16, kind="ExternalInput").ap()
    wvT_d = nc.dram_tensor("wvT", (D, D), BF16, kind="ExternalInput").ap()
    wpT_d = nc.dram_tensor("wpT", (D, D), BF16, kind="ExternalInput").ap()
    bp_d = nc.dram_tensor("bp", (1, D), BF16, kind="ExternalInput").ap()
    mask_d = nc.dram_tensor("mask", (P, kt), F32, kind="ExternalInput").ap()
    wqc_d = nc.dram_tensor("wqc", (P, 1), F32, kind="ExternalInput").ap()
    wkc_d = nc.dram_tensor("wkc", (P, 1), F32, kind="ExternalInput").ap()
    out_d = nc.dram_tensor("out", (LQ, D), F32, kind="ExternalOutput").ap()

    with tile.TileContext(nc) as tc:
        _kernel_body(
            nc, tc, lkv, xT_d, ctxT_d, wqT_d, wkT_d, wvT_d, wpT_d, bp_d, mask_d,
            wqc_d, wkc_d, out_d,
        )
    nc.compile()
    return nc


def _kernel_body(
    nc, tc, lkv, xT_d, ctxT_d, wqT_d, wkT_d, wvT_d, wpT_d, bp_d, mask_d,
    wqc_d, wkc_d, out_d,
):
    import contextlib

    kt = lkv // P

    ctx = contextlib.ExitStack()
    with ctx:
        const = ctx.enter_context(tc.tile_pool(name="const", bufs=1))
        wpool = ctx.enter_context(tc.tile_pool(name="wpool", bufs=1))
        xio = ctx.enter_context(tc.tile_pool(name="xio", bufs=1))
        dbuf = ctx.enter_context(tc.tile_pool(name="dbuf", bufs=2))
        mm_ps = ctx.enter_context(tc.tile_pool(name="mm_ps", bufs=2, space="PSUM"))
        sc_ps = ctx.enter_context(tc.tile_pool(name="sc_ps", bufs=2, space="PSUM"))
        pv_ps = ctx.enter_context(tc.tile_pool(name="pv_ps", bufs=1, space="PSUM"))
        dram = ctx.enter_context(tc.tile_pool(name="dram", bufs=2, space="DRAM"))
        work = ctx.enter_context(tc.tile_pool(name="work", bufs=1))
        sq_pool = ctx.enter_context(tc.tile_pool(name="sq", bufs=1))
        # separate slot groups for the q/k-norm broadcasts vs the softmax
        # normalize broadcasts: a shared group serializes chunk c+1's
        # q-norm behind chunk c's normalize, breaking the pipeline
        rbq_pool = ctx.enter_context(tc.tile_pool(name="rbq", bufs=2))
        rbn_pool = ctx.enter_context(tc.tile_pool(name="rbn", bufs=2))
        probs_pool = ctx.enter_context(tc.tile_pool(name="probs", bufs=3))
        out_pool = ctx.enter_context(tc.tile_pool(name="outp", bufs=2))
        rowp = ctx.enter_context(tc.tile_pool(name="rowp", bufs=1))
        ab_pool = ctx.enter_context(tc.tile_pool(name="ab", bufs=1))

        # ---- small constants ----
        mask_sb = const.tile([P, kt], F32, name="mask_sb")
        nc.sync.dma_start(mask_sb[:], mask_d[:])
        wqc_sb = const.tile([P, 1], F32, name="wqc_sb")
        nc.sync.dma_start(wqc_sb[:], wqc_d[:])
        wkc_sb = const.tile([P, 1], F32, name="wkc_sb")
        nc.sync.dma_start(wkc_sb[:], wkc_d[:])
        bp_sb = const.tile([1, D], BF16, name="bp_sb")
        nc.sync.dma_start(bp_sb[:], bp_d[:])
        ones_row = const.tile([1, P], BF16, name="ones_row")
        nc.vector.memset(ones_row[:], 1.0)
        eps16 = const.tile([16, 1], F32, name="eps16")
        nc.vector.memset(eps16[:], EPS)
        zero16 = const.tile([16, 1], F32, name="zero16")
        nc.vector.memset(zero16[:], 0.0)
        sel16 = []
        for m in range(DT):
            s = const.tile([P, 16], BF16, name=f"sel{m}")
            nc.vector.memset(s[:], 0.0)
            nc.vector.memset(s[0:64, 2 * m : 2 * m + 1], 1.0)
            nc.vector.memset(s[64:128, 2 * m + 1 : 2 * m + 2], 1.0)
            sel16.append(s)

        khat = [const.tile([P, lkv], BF16, name=f"khat{m}") for m in range(DT)]
        vsb = [const.tile([P, H * (HD + 1)], BF16, name=f"vsb{m}") for m in range(kt)]

        # ---- ctx + stage-A weights (Wk, Wv) ----
        # (the ctx{k} slot groups are reused by attn(0); slots size to max)
        ctx_sb = []
        for k in range(DT):
            t = xio.tile([P, lkv], BF16, name=f"ctx{k}", tag=f"ctx{k}")
            nc.sync.dma_start(t[:], ctxT_d[P * k : P * (k + 1), :])
            ctx_sb.append(t)
        wk_sb, wv_sb = [], []
        for k in range(DT):
            t = wpool.tile([P, D], BF16, name=f"wk{k}", tag=f"wa{k}")
            nc.sync.dma_start(t[:], wkT_d[P * k : P * (k + 1), :])
            wk_sb.append(t)
            t = wpool.tile([P, D], BF16, name=f"wv{k}", tag=f"wb{k}")
            nc.sync.dma_start(t[:], wvT_d[P * k : P * (k + 1), :])
            wv_sb.append(t)

        def bcast_rows(rs_sb, ncols, wcol, nm, pool, tag):
            """(16, ncols) bf16 rows -> per-d-tile (128, ncols) bf16 tiles:
            row 2m+j broadcast to partitions 64j..64j+63, times wcol[p]."""
            bounce = dram.tile([16, ncols], BF16, name=f"dr_{nm}", tag=f"dr_{nm}")
            nc.sync.dma_start(bounce[:], rs_sb[:])
            tiles = []
            for m in range(DT):
                rb = pool.tile([P, ncols], BF16, name=f"rb_{nm}{m}", tag=tag)
                for j in range(2):
                    nc.sync.dma_start(
                        rb[64 * j : 64 * (j + 1), :],
                        bounce[2 * m + j : 2 * m + j + 1, :].broadcast_to((64, ncols)),
                    )
                if wcol is not None:
                    nc.vector.tensor_scalar(
                        rb[:], rb[:], wcol[:], None, mybir.AluOpType.mult
                    )
                tiles.append(rb)
            return tiles

        def project(dst_tiles, w_tiles, act_tiles, ncols, scalar_col, nm):
            """dst[m][:, :] (bf16) = (W @ act) for d'-tile m, then RMS-norm
            applied in place via selector-matmul stats + Ln/Exp + broadcast."""
            for m in range(DT):
                for o, w in _slices(ncols):
                    ps = mm_ps.tile([P, 512], F32, name=f"ps_{nm}", tag="mm")
                    for k in range(DT):
                        nc.tensor.matmul(
                            ps[:, 0:w],
                            w_tiles[k][:, P * m : P * (m + 1)],
                            act_tiles[k][:, o : o + w],
                            start=(k == 0), stop=(k == DT - 1),
                        )
                    nc.vector.tensor_copy(
                        dst_tiles[m][:, o : o + w], ps[:, 0:w]
                    )
            sq_tiles = []
            for m in range(DT):
                sq = sq_pool.tile([P, ncols], BF16, name=f"sq_{nm}{m}", tag=f"sq{m}")
                nc.vector.tensor_tensor(
                    sq[:], dst_tiles[m][:, 0:ncols], dst_tiles[m][:, 0:ncols],
                    mybir.AluOpType.mult
                )
                sq_tiles.append(sq)
            ln_t = work.tile([16, ncols], F32, name=f"ln_{nm}", tag="ln")
            for o, w in _slices(ncols):
                st = mm_ps.tile([16, 512], F32, name=f"stp_{nm}", tag="mm")
                for m in range(DT):
                    nc.tensor.matmul(
                        st[:, 0:w], sel16[m][:], sq_tiles[m][:, o : o + w],
                        start=(m == 0), stop=(m == DT - 1),
                    )
                nc.scalar.activation(
                    ln_t[:, o : o + w], st[:, 0:w],
                    mybir.ActivationFunctionType.Ln,
                    bias=eps16[:], scale=1.0 / HD,
                )
            rs = work.tile([16, ncols], BF16, name=f"rs_{nm}", tag="rs")
            nc.scalar.activation(
                rs[:], ln_t[:], mybir.ActivationFunctionType.Exp,
                bias=zero16[:], scale=-0.5,
            )
            rbt = bcast_rows(rs, ncols, scalar_col, nm, rbq_pool, "rbq")
            for m in range(DT):
                nc.vector.tensor_tensor(
                    dst_tiles[m][:], dst_tiles[m][:], rbt[m][:], mybir.AluOpType.mult
                )

        # ================= K / V stage =================
        project(khat, wk_sb, ctx_sb, lkv, wkc_sb, "k")

        for m in range(kt):
            for n in range(2):
                ps = mm_ps.tile([P, 512], F32, name="ps_v", tag="mm")
                for k in range(DT):
                    nc.tensor.matmul(
                        ps[:],
                        ctx_sb[k][:, P * m : P * (m + 1)],
                        wv_sb[k][:, 512 * n : 512 * (n + 1)],
                        start=(k == 0), stop=(k == DT - 1),
                    )
                v3 = vsb[m][:].rearrange("p (h e) -> p h e", e=HD + 1)
                nc.vector.tensor_copy(
                    v3[:, 8 * n : 8 * (n + 1), 0:HD],
                    ps[:].rearrange("p (h e) -> p h e", e=HD),
                )
            v3 = vsb[m][:].rearrange("p (h e) -> p h e", e=HD + 1)
            nc.vector.memset(v3[:, :, HD : HD + 1], 1.0)

        # ---- stage-B weights (Wq, Wp) reuse the Wk/Wv SBUF slots ----
        wq_sb, wp_sb = [], []
        for k in range(DT):
            t = wpool.tile([P, D], BF16, name=f"wq{k}", tag=f"wa{k}")
            nc.sync.dma_start(t[:], wqT_d[P * k : P * (k + 1), :])
            wq_sb.append(t)
            t = wpool.tile([P, D], BF16, name=f"wp{k}", tag=f"wb{k}")
            nc.sync.dma_start(t[:], wpT_d[P * k : P * (k + 1), :])
            wp_sb.append(t)

        # ================= per-chunk pipeline ==========================
        # chunk c: Q-proj (fills PE during attn(c-1)), attention, normalize;
        # out-proj of chunk c-1 is emitted interleaved into attention(c).
        prev = None  # (attn tiles, chunk idx) pending out-projection

        def emit_outproj(attn_tiles, c, m):
            """out rows [c*CH + m*128, +128) = attnT @ WpT + bias."""
            for n in range(2):
                ps = mm_ps.tile([P, 512], F32, name="ps_o", tag="mm")
                for k in range(DT):
                    nc.tensor.matmul(
                        ps[:],
                        attn_tiles[k][:, P * m : P * (m + 1)],
                        wp_sb[k][:, 512 * n : 512 * (n + 1)],
                        start=(k == 0), stop=False,
                    )
                nc.tensor.matmul(
                    ps[:], ones_row[:], bp_sb[:, 512 * n : 512 * (n + 1)],
                    start=False, stop=True,
                )
                o_sb = out_pool.tile([P, 512], F32, name="o_sb", tag="o")
                nc.vector.tensor_copy(o_sb[:], ps[:])
                nc.sync.dma_start(
                    out_d[CH * c + P * m : CH * c + P * (m + 1),
                          512 * n : 512 * (n + 1)],
                    o_sb[:],
                )

        for c in range(NCH):
            x_sb = []
            for k in range(DT):
                t = xio.tile([P, CH], BF16, name=f"x{k}_{c}", tag=f"x{k}")
                nc.sync.dma_start(t[:], xT_d[P * k : P * (k + 1), CH * c : CH * (c + 1)])
                x_sb.append(t)

            qhat = [dbuf.tile([P, CH], BF16, name=f"qhat{m}_{c}", tag=f"qhat{m}")
                    for m in range(DT)]
            project(qhat, wq_sb, x_sb, CH, wqc_sb, f"q{c}")

            # chunk 0's attn tiles reuse the (dead after V-proj) ctx slots
            if c == 0:
                attn = [xio.tile([P, CH], BF16, name=f"attn{m}_0", tag=f"ctx{m}")
                        for m in range(DT)]
            else:
                attn = [ab_pool.tile([P, CH], BF16, name=f"attn{m}_{c}",
                                     tag=f"attn{m}")
                        for m in range(DT)]
            den = work.tile([16, CH], F32, name=f"den{c}", tag=f"den{c % 2}")

            for h in range(H):
                mt, off = h // 2, 64 * (h % 2)
                pv = pv_ps.tile([HD + 1, CH], F32, name="pv", tag="pv")
                for t in range(kt):
                    sc = sc_ps.tile([P, CH], F32, name="sc", tag="sc")
                    for n in range(NS):
                        nc.tensor.matmul(
                            sc[:, 512 * n : 512 * (n + 1)],
                            khat[mt][off : off + HD, P * t : P * (t + 1)],
                            qhat[mt][off : off + HD, 512 * n : 512 * (n + 1)],
                            start=True, stop=True,
                        )
                    pr = probs_pool.tile([P, CH], BF16, name="pr", tag="pr")
                    nc.scalar.activation(
                        pr[:], sc[:], mybir.ActivationFunctionType.Exp,
                        bias=mask_sb[:, t : t + 1], scale=1.0,
                    )
                    for n in range(NS):
                        nc.tensor.matmul(
                            pv[:, 512 * n : 512 * (n + 1)],
                            vsb[t][:, (HD + 1) * h : (HD + 1) * (h + 1)],
                            pr[:, 512 * n : 512 * (n + 1)],
                            start=(t == 0), stop=(t == kt - 1),
                        )
                nc.vector.tensor_copy(attn[mt][off : off + HD, :], pv[0:HD, :])
                dnr = rowp.tile([1, CH], F32, name="dnr", tag="dnr")
                nc.vector.tensor_copy(dnr[:], pv[HD : HD + 1, :])
                nc.sync.dma_start(den[h : h + 1, :], dnr[:])
                # interleave previous chunk's out-projection into this
                # (ACT-bound) attention window to keep the PE busy:
                # CH/P = 8 q-subtiles spread over 16 heads -> one per even head
                if prev is not None and h % 2 == 0:
                    pattn, pc = prev
                    emit_outproj(pattn, pc, h // 2)
            # ---- normalize: recip(denominator) broadcast multiply ----
            nc.vector.reciprocal(den[:], den[:])
            rcpbf = work.tile([16, CH], BF16, name=f"rcp{c}", tag="rcpbf")
            nc.vector.tensor_copy(rcpbf[:], den[:])
            rbt = bcast_rows(rcpbf, CH, None, f"n{c}", rbn_pool, "rbn")
            for m in range(DT):
                nc.vector.tensor_tensor(
                    attn[m][:], attn[m][:], rbt[m][:], mybir.AluOpType.mult
                )
            prev = (attn, c)

        # final chunk's out-projection (nothing left to overlap with)
        pattn, pc = prev
        for m in range(CH // P):
            emit_outproj(pattn, pc, m)


def _prep_inputs(x, context, context_mask, Wq, Wk, Wv, Wp, bp, q_norm_w, k_norm_w):
    """Compact each batch's context to its valid (unmasked) kv tokens,
    padded to a common multiple of 128. Softmax over the compacted set is
    mathematically identical (masked tokens contribute exactly 0)."""
    scale = HD ** -0.5
    shared = {
        "wqT": np.ascontiguousarray(Wq.T).astype(BFNP),
        "wkT": np.ascontiguousarray(Wk.T).astype(BFNP),
        "wvT": np.ascontiguousarray(Wv.T).astype(BFNP),
        "wpT": np.ascontiguousarray(Wp.T).astype(BFNP),
        "bp": bp.reshape(1, D).astype(BFNP),
        "wqc": np.tile(q_norm_w.astype(np.float64) * scale, 2)
        .reshape(P, 1).astype(np.float32),
        "wkc": np.tile(k_norm_w, 2).reshape(P, 1).astype(np.float32),
    }
    idxs = []
    for b in range(B):
        m = context_mask[b].astype(bool).copy()
        if not m.any():
            m[0] = True
        idxs.append(np.nonzero(m)[0])
    lkv = max(128, -(-max(len(i) for i in idxs) // P) * P)
    kt = lkv // P
    in_maps = []
    for b in range(B):
        idx = idxs[b]
        ctx_c = np.zeros((lkv, D), np.float32)
        ctx_c[: len(idx)] = context[b][idx]
        bias = np.full(lkv, NEG, np.float32)
        bias[: len(idx)] = 0.0
        in_maps.append(
            dict(
                shared,
                xT=np.ascontiguousarray(x[b].T).astype(BFNP),
                ctxT=np.ascontiguousarray(ctx_c.T).astype(BFNP),
                mask=np.ascontiguousarray(bias.reshape(kt, P).T),
            )
        )
    return lkv, in_maps


def kernel(x, context, context_mask, Wq, Wk, Wv, Wp, bp, q_norm_w, k_norm_w):
    global LAST_RESULTS
    x = np.asarray(x, dtype=np.float32)
    context = np.asarray(context, dtype=np.float32)
    context_mask = np.asarray(context_mask)
    lkv, in_maps = _prep_inputs(
        x, context, context_mask,
        np.asarray(Wq, np.float32), np.asarray(Wk, np.float32),
        np.asarray(Wv, np.float32), np.asarray(Wp, np.float32),
        np.asarray(bp, np.float32), np.asarray(q_norm_w, np.float32),
        np.asarray(k_norm_w, np.float32),
    )
    if lkv not in _CACHE:
        _CACHE[lkv] = _build(lkv)
    nc = _CACHE[lkv]
    res = bass_utils.run_bass_kernel_spmd(nc, in_maps, core_ids=list(range(B)))
    LAST_RESULTS = res
    return np.stack([res.results[b]["out"] for b in range(B)], axis=0)


# revision 26
# speedup vs baseline: 1.5796x; 1.0081x over previous
"""Trainium2 Bass kernel for nn_CrossAttention.

Sharding: data-parallel over batch (B=8 -> 8 cores, one batch element per
core). No collectives. Host pre-transposes activations/weights into
contraction-on-partition layouts and casts to bf16; all matmuls run at
1 cyc/row on the PE with fp32 PSUM accumulation.

Per-core dataflow (batch b):
  QT  = WqT-matmuls over xT       -> (d', lq)  "transposed" layout
  KT  = WkT-matmuls over ctxT     -> (d', lkv)
  V   = ctxT-matmuls over WvT     -> (lkv, d') + ones column per head
  RMS-norm factors per (head, pos) via PE selector matmuls + Ln/Exp,
     applied through DRAM-bounce partition broadcasts.
  scoresT = khat.T @ qhat per head -> (lkv, lq); exp on ACT with the
     context-mask as per-partition bias; probsT in bf16.
  PV: V (with ones col) @ probsT -> (hd+1, lq): row 64 is the softmax
     denominator. Normalize via DVE reciprocal + broadcast multiply.
  out = attnT-matmuls over WpT + bias-row matmul -> (lq, d') fp32.

Pipelining: qhat/attn are double-buffered so chunk c+1's Q-projection
fills PE bubbles during chunk c's (ACT-bound) attention, and chunk c's
out-projection is emitted interleaved into chunk c+1's attention heads.
Weight SBUF slots are reused (Wk->Wq, Wv->Wp).
"""

import sys

for _p in ("/opt/trn_rl_repo",):
    if _p not in sys.path:
        sys.path.insert(0, _p)

import numpy as np
import ml_dtypes

import concourse.bass as bass
import concourse.mybir as mybir
import concourse.tile as tile
from concourse import bacc
from concourse import bass_utils

BF16 = mybir.dt.bfloat16
F32 = mybir.dt.float32
BFNP = ml_dtypes.bfloat16

B, LQ, LKV, D, H = 8, 2048, 1024, 1024, 16
HD = D // H          # 64
P = 128              # partitions
DT = D // P          # 8 d-tiles
KT_ = LKV // P       # 8 lkv-tiles
CH = 1024            # lq chunk
NCH = LQ // CH       # 2
NS = CH // 512       # 512-wide matmul slices per chunk
EPS = 1e-6
NEG = -1.0e30

_CACHE = {}
LAST_RESULTS = None


def _patch_act_tables():
    """Restrict usable ACT function sets to natural_log_exp_and_others (it
    contains both Exp and Ln) so the table-load pass never alternates between
    exp_and_others / natural_log — each switch costs ~2.7us on ScalarE.
    Indices (act_func_set_id) are preserved; other sets are just emptied."""
    import concourse.hw_specs as hw_specs
    import concourse.bass_interp as bass_interp

    if getattr(_patch_act_tables, "_done", False):
        return
    orig = hw_specs.get_activation_tables

    def patched(module_arch):
        t = orig(module_arch)
        keep = "natural_log_exp_and_others"
        if keep in t:
            t = {k: (v if k == keep else set()) for k, v in t.items()}
        return t

    hw_specs.get_activation_tables = patched
    bacc.get_activation_tables = patched
    bass_interp.get_activation_tables = patched
    _patch_act_tables._done = True


def _slices(ncols):
    """(offset, width) pairs covering ncols in <=512-wide matmul slices."""
    out, o = [], 0
    while o < ncols:
        w = min(512, ncols - o)
        out.append((o, w))
        o += w
    return out


def _build(lkv):
    """Compile for a compacted context length lkv (multiple of 128)."""
    _patch_act_tables()
    nc = bacc.Bacc("TRN2", target_bir_lowering=False, debug=False)

    kt = lkv // P
    xT_d = nc.dram_tensor("xT", (D, LQ), BF16, kind="ExternalInput").ap()
    ctxT_d = nc.dram_tensor("ctxT", (D, lkv), BF16, kind="ExternalInput").ap()
    wqT_d = nc.dram_tensor("wqT", (D, D), BF16, kind="ExternalInput").ap()
    wkT_d = nc.dram_tensor("wkT", (D, D), BF16, kind="ExternalInput").ap()
    wvT_d = nc.dram_tensor("wvT", (D, D), BF16, kind="ExternalInput").ap()
    wpT_d = nc.dram_tensor("wpT", (D, D), BF16, kind="ExternalInput").ap()
    bp_d = nc.dram_tensor("bp", (1, D), BF16, kind="ExternalInput").ap()
    mask_d = nc.dram_tensor("mask", (P, kt), F32, kind="ExternalInput").ap()
    wqc_d = nc.dram_tensor("wqc", (P, 1), F32, kind="ExternalInput").ap()
    wkc_d = nc.dram_tensor("wkc", (P, 1), F32, kind="ExternalInput").ap()
    out_d = nc.dram_tensor("out", (LQ, D), F32, kind="ExternalOutput").ap()

    with tile.TileContext(nc) as tc:
        _kernel_body(
            nc, tc, lkv, xT_d, ctxT_d, wqT_d, wkT_d, wvT_d, wpT_d, bp_d, mask_d,
            wqc_d, wkc_d, out_d,
        )
    nc.compile()
    return nc


def _kernel_body(
    nc, tc, lkv, xT_d, ctxT_d, wqT_d, wkT_d, wvT_d, wpT_d, bp_d, mask_d,
    wqc_d, wkc_d, out_d,
):
    import contextlib

    kt = lkv // P

    ctx = contextlib.ExitStack()
    with ctx:
        const = ctx.enter_context(tc.tile_pool(name="const", bufs=1))
        wpool = ctx.enter_context(tc.tile_pool(name="wpool", bufs=1))
        xio = ctx.enter_context(tc.tile_pool(name="xio", bufs=1))
        dbuf = ctx.enter_context(tc.tile_pool(name="dbuf", bufs=2))
        mm_ps = ctx.enter_context(tc.tile_pool(name="mm_ps", bufs=2, space="PSUM"))
        sc_ps = ctx.enter_context(tc.tile_pool(name="sc_ps", bufs=2, space="PSUM"))
        pv_ps = ctx.enter_context(tc.tile_pool(name="pv_ps", bufs=1, space="PSUM"))
        dram = ctx.enter_context(tc.tile_pool(name="dram", bufs=2, space="DRAM"))
        work = ctx.enter_context(tc.tile_pool(name="work", bufs=1))
        sq_pool = ctx.enter_context(tc.tile_pool(name="sq", bufs=1))
        # separate slot groups for the q/k-norm broadcasts vs the softmax
        # normalize broadcasts: a shared group serializes chunk c+1's
        # q-norm behind chunk c's normalize, breaking the pipeline
        rbq_pool = ctx.enter_context(tc.tile_pool(name="rbq", bufs=2))
        rbn_pool = ctx.enter_context(tc.tile_pool(name="rbn", bufs=2))
        probs_pool = ctx.enter_context(tc.tile_pool(name="probs", bufs=3))
        out_pool = ctx.enter_context(tc.tile_pool(name="outp", bufs=2))
        rowp = ctx.enter_context(tc.tile_pool(name="rowp", bufs=1))
        ab_pool = ctx.enter_context(tc.tile_pool(name="ab", bufs=1))

        # ---- small constants ----
        mask_sb = const.tile([P, kt], F32, name="mask_sb")
        nc.sync.dma_start(mask_sb[:], mask_d[:])
        wqc_sb = const.tile([P, 1], F32, name="wqc_sb")
        nc.sync.dma_start(wqc_sb[:], wqc_d[:])
        wkc_sb = const.tile([P, 1], F32, name="wkc_sb")
        nc.sync.dma_start(wkc_sb[:], wkc_d[:])
        bp_sb = const.tile([1, D], BF16, name="bp_sb")
        nc.sync.dma_start(bp_sb[:], bp_d[:])
        ones_row = const.tile([1, P], BF16, name="ones_row")
        nc.vector.memset(ones_row[:], 1.0)
        eps16 = const.tile([16, 1], F32, name="eps16")
        nc.vector.memset(eps16[:], EPS)
        zero16 = const.tile([16, 1], F32, name="zero16")
        nc.vector.memset(zero16[:], 0.0)
        sel16 = []
        for m in range(DT):
            s = const.tile([P, 16], BF16, name=f"sel{m}")
            nc.vector.memset(s[:], 0.0)
            nc.vector.memset(s[0:64, 2 * m : 2 * m + 1], 1.0)
            nc.vector.memset(s[64:128, 2 * m + 1 : 2 * m + 2], 1.0)
            sel16.append(s)

        khat = [const.tile([P, lkv], BF16, name=f"khat{m}") for m in range(DT)]
        vsb = [const.tile([P, H * (HD + 1)], BF16, name=f"vsb{m}") for m in range(kt)]

        # ---- HAM warmup: dummy matmuls spanning the initial ctx/weight DMA
        # so the PE clock is at 8/8 when the first projection issues ----
        wz = const.tile([P, 512], BF16, name="wz")
        nc.vector.memset(wz[:], 0.0)
        for i in range(24):
            psw = mm_ps.tile([16, 512], F32, name="psw", tag="mm")
            nc.tensor.matmul(psw[:], sel16[0][:], wz[:], start=True, stop=True)

        # ---- ctx + stage-A weights (Wk, Wv) ----
        # (the ctx{k} slot groups are reused by attn(0); slots size to max)
        ctx_sb = []
        for k in range(DT):
            t = xio.tile([P, lkv], BF16, name=f"ctx{k}", tag=f"ctx{k}")
            nc.sync.dma_start(t[:], ctxT_d[P * k : P * (k + 1), :])
            ctx_sb.append(t)
        wk_sb, wv_sb = [], []
        for k in range(DT):
            t = wpool.tile([P, D], BF16, name=f"wk{k}", tag=f"wa{k}")
            nc.sync.dma_start(t[:], wkT_d[P * k : P * (k + 1), :])
            wk_sb.append(t)
            t = wpool.tile([P, D], BF16, name=f"wv{k}", tag=f"wb{k}")
            nc.sync.dma_start(t[:], wvT_d[P * k : P * (k + 1), :])
            wv_sb.append(t)

        def bcast_rows(rs_sb, ncols, wcol, nm, pool, tag):
            """(16, ncols) bf16 rows -> per-d-tile (128, ncols) bf16 tiles:
            row 2m+j broadcast to partitions 64j..64j+63, times wcol[p]."""
            bounce = dram.tile([16, ncols], BF16, name=f"dr_{nm}", tag=f"dr_{nm}")
            nc.sync.dma_start(bounce[:], rs_sb[:])
            tiles = []
            for m in range(DT):
                rb = pool.tile([P, ncols], BF16, name=f"rb_{nm}{m}", tag=tag)
                for j in range(2):
                    nc.sync.dma_start(
                        rb[64 * j : 64 * (j + 1), :],
                        bounce[2 * m + j : 2 * m + j + 1, :].broadcast_to((64, ncols)),
                    )
                if wcol is not None:
                    nc.vector.tensor_scalar(
                        rb[:], rb[:], wcol[:], None, mybir.AluOpType.mult
                    )
                tiles.append(rb)
            return tiles

        def project(dst_tiles, w_tiles, act_tiles, ncols, scalar_col, nm):
            """dst[m][:, :] (bf16) = (W @ act) for d'-tile m, then RMS-norm
            applied in place via selector-matmul stats + Ln/Exp + broadcast."""
            for m in range(DT):
                for o, w in _slices(ncols):
                    ps = mm_ps.tile([P, 512], F32, name=f"ps_{nm}", tag="mm")
                    for k in range(DT):
                        nc.tensor.matmul(
                            ps[:, 0:w],
                            w_tiles[k][:, P * m : P * (m + 1)],
                            act_tiles[k][:, o : o + w],
                            start=(k == 0), stop=(k == DT - 1),
                        )
                    nc.vector.tensor_copy(
                        dst_tiles[m][:, o : o + w], ps[:, 0:w]
                    )
            sq_tiles = []
            for m in range(DT):
                sq = sq_pool.tile([P, ncols], BF16, name=f"sq_{nm}{m}", tag=f"sq{m}")
                nc.vector.tensor_tensor(
                    sq[:], dst_tiles[m][:, 0:ncols], dst_tiles[m][:, 0:ncols],
                    mybir.AluOpType.mult
                )
                sq_tiles.append(sq)
            ln_t = work.tile([16, ncols], F32, name=f"ln_{nm}", tag="ln")
            for o, w in _slices(ncols):
                st = mm_ps.tile([16, 512], F32, name=f"stp_{nm}", tag="mm")
                for m in range(DT):
                    nc.tensor.matmul(
                        st[:, 0:w], sel16[m][:], sq_tiles[m][:, o : o + w],
                        start=(m == 0), stop=(m == DT - 1),
                    )
                nc.scalar.activation(
                    ln_t[:, o : o + w], st[:, 0:w],
                    mybir.ActivationFunctionType.Ln,
                    bias=eps16[:], scale=1.0 / HD,
                )
            rs = work.tile([16, ncols], BF16, name=f"rs_{nm}", tag="rs")
            nc.scalar.activation(
                rs[:], ln_t[:], mybir.ActivationFunctionType.Exp,
                bias=zero16[:], scale=-0.5,
            )
            rbt = bcast_rows(rs, ncols, scalar_col, nm, rbq_pool, "rbq")
            for m in range(DT):
                nc.vector.tensor_tensor(
                    dst_tiles[m][:], dst_tiles[m][:], rbt[m][:], mybir.AluOpType.mult
                )

        # ================= K / V stage =================
        project(khat, wk_sb, ctx_sb, lkv, wkc_sb, "k")

        for m in range(kt):
            for n in range(2):
                ps = mm_ps.tile([P, 512], F32, name="ps_v", tag="mm")
                for k in range(DT):
                    nc.tensor.matmul(
                        ps[:],
                        ctx_sb[k][:, P * m : P * (m + 1)],
                        wv_sb[k][:, 512 * n : 512 * (n + 1)],
                        start=(k == 0), stop=(k == DT - 1),
                    )
                v3 = vsb[m][:].rearrange("p (h e) -> p h e", e=HD + 1)
                nc.vector.tensor_copy(
                    v3[:, 8 * n : 8 * (n + 1), 0:HD],
                    ps[:].rearrange("p (h e) -> p h e", e=HD),
                )
            v3 = vsb[m][:].rearrange("p (h e) -> p h e", e=HD + 1)
            nc.vector.memset(v3[:, :, HD : HD + 1], 1.0)

        # ---- stage-B weights (Wq, Wp) reuse the Wk/Wv SBUF slots ----
        wq_sb, wp_sb = [], []
        for k in range(DT):
            t = wpool.tile([P, D], BF16, name=f"wq{k}", tag=f"wa{k}")
            nc.sync.dma_start(t[:], wqT_d[P * k : P * (k + 1), :])
            wq_sb.append(t)
            t = wpool.tile([P, D], BF16, name=f"wp{k}", tag=f"wb{k}")
            nc.sync.dma_start(t[:], wpT_d[P * k : P * (k + 1), :])
            wp_sb.append(t)

        # ================= per-chunk pipeline ==========================
        # chunk c: Q-proj (fills PE during attn(c-1)), attention, normalize;
        # out-proj of chunk c-1 is emitted interleaved into attention(c).
        prev = None  # (attn tiles, chunk idx) pending out-projection

        def emit_outproj(attn_tiles, c, m):
            """out rows [c*CH + m*128, +128) = attnT @ WpT + bias."""
            for n in range(2):
                ps = mm_ps.tile([P, 512], F32, name="ps_o", tag="mm")
                for k in range(DT):
                    nc.tensor.matmul(
                        ps[:],
                        attn_tiles[k][:, P * m : P * (m + 1)],
                        wp_sb[k][:, 512 * n : 512 * (n + 1)],
                        start=(k == 0), stop=False,
                    )
                nc.tensor.matmul(
                    ps[:], ones_row[:], bp_sb[:, 512 * n : 512 * (n + 1)],
                    start=False, stop=True,
                )
                o_sb = out_pool.tile([P, 512], F32, name="o_sb", tag="o")
                nc.vector.tensor_copy(o_sb[:], ps[:])
                nc.sync.dma_start(
                    out_d[CH * c + P * m : CH * c + P * (m + 1),
                          512 * n : 512 * (n + 1)],
                    o_sb[:],
                )

        for c in range(NCH):
            x_sb = []
            for k in range(DT):
                t = xio.tile([P, CH], BF16, name=f"x{k}_{c}", tag=f"x{k}")
                nc.sync.dma_start(t[:], xT_d[P * k : P * (k + 1), CH * c : CH * (c + 1)])
                x_sb.append(t)

            qhat = [dbuf.tile([P, CH], BF16, name=f"qhat{m}_{c}", tag=f"qhat{m}")
                    for m in range(DT)]
            project(qhat, wq_sb, x_sb, CH, wqc_sb, f"q{c}")

            # chunk 0's attn tiles reuse the (dead after V-proj) ctx slots
            if c == 0:
                attn = [xio.tile([P, CH], BF16, name=f"attn{m}_0", tag=f"ctx{m}")
                        for m in range(DT)]
            else:
                attn = [ab_pool.tile([P, CH], BF16, name=f"attn{m}_{c}",
                                     tag=f"attn{m}")
                        for m in range(DT)]
            den = work.tile([16, CH], F32, name=f"den{c}", tag=f"den{c % 2}")

            for h in range(H):
                mt, off = h // 2, 64 * (h % 2)
                pv = pv_ps.tile([HD + 1, CH], F32, name="pv", tag="pv")
                for t in range(kt):
                    sc = sc_ps.tile([P, CH], F32, name="sc", tag="sc")
                    for n in range(NS):
                        nc.tensor.matmul(
                            sc[:, 512 * n : 512 * (n + 1)],
                            khat[mt][off : off + HD, P * t : P * (t + 1)],
                            qhat[mt][off : off + HD, 512 * n : 512 * (n + 1)],
                            start=True, stop=True,
                        )
                    pr = probs_pool.tile([P, CH], BF16, name="pr", tag="pr")
                    nc.scalar.activation(
                        pr[:], sc[:], mybir.ActivationFunctionType.Exp,
                        bias=mask_sb[:, t : t + 1], scale=1.0,
                    )
                    for n in range(NS):
                        nc.tensor.matmul(
                            pv[:, 512 * n : 512 * (n + 1)],
                            vsb[t][:, (HD + 1) * h : (HD + 1) * (h + 1)],
                            pr[:, 512 * n : 512 * (n + 1)],
                            start=(t == 0), stop=(t == kt - 1),
                        )
                nc.vector.tensor_copy(attn[mt][off : off + HD, :], pv[0:HD, :])
                dnr = rowp.tile([1, CH], F32, name="dnr", tag="dnr")
                nc.vector.tensor_copy(dnr[:], pv[HD : HD + 1, :])
                nc.sync.dma_start(den[h : h + 1, :], dnr[:])
                # interleave previous chunk's out-projection into this
                # attention window to keep the PE busy; reserve the last two
                # q-subtiles to bridge the PE gap during the normalize chain
                if prev is not None and h % 2 == 0 and h < 12:
                    pattn, pc = prev
                    emit_outproj(pattn, pc, h // 2)
            # reserved bridge tiles of the previous out-projection: these are
            # ready instantly, so the PE chews them while the normalize
            # chain below runs on DVE/DMA (avoids a >3.4us HAM re-throttle)
            if prev is not None:
                pattn, pc = prev
                emit_outproj(pattn, pc, 6)
                emit_outproj(pattn, pc, 7)
            # ---- normalize: recip(denominator) broadcast multiply ----
            nc.vector.reciprocal(den[:], den[:])
            rcpbf = work.tile([16, CH], BF16, name=f"rcp{c}", tag="rcpbf")
            nc.vector.tensor_copy(rcpbf[:], den[:])
            rbt = bcast_rows(rcpbf, CH, None, f"n{c}", rbn_pool, "rbn")
            for m in range(DT):
                nc.vector.tensor_tensor(
                    attn[m][:], attn[m][:], rbt[m][:], mybir.AluOpType.mult
                )
            prev = (attn, c)

        # final chunk's out-projection (nothing left to overlap with)
        pattn, pc = prev
        for m in range(CH // P):
            emit_outproj(pattn, pc, m)


def _prep_inputs(x, context, context_mask, Wq, Wk, Wv, Wp, bp, q_norm_w, k_norm_w):
    """Compact each batch's context to its valid (unmasked) kv tokens,
    padded to a common multiple of 128. Softmax over the compacted set is
    mathematically identical (masked tokens contribute exactly 0)."""
    scale = HD ** -0.5
    shared = {
        "wqT": np.ascontiguousarray(Wq.T).astype(BFNP),
        "wkT": np.ascontiguousarray(Wk.T).astype(BFNP),
        "wvT": np.ascontiguousarray(Wv.T).astype(BFNP),
        "wpT": np.ascontiguousarray(Wp.T).astype(BFNP),
        "bp": bp.reshape(1, D).astype(BFNP),
        "wqc": np.tile(q_norm_w.astype(np.float64) * scale, 2)
        .reshape(P, 1).astype(np.float32),
        "wkc": np.tile(k_norm_w, 2).reshape(P, 1).astype(np.float32),
    }
    idxs = []
    for b in range(B):
        m = context_mask[b].astype(bool).copy()
        if not m.any():
            m[0] = True
        idxs.append(np.nonzero(m)[0])
    lkv = max(128, -(-max(len(i) for i in idxs) // P) * P)
    kt = lkv // P
    in_maps = []
    for b in range(B):
        idx = idxs[b]
        ctx_c = np.zeros((lkv, D), np.float32)
        ctx_c[: len(idx)] = context[b][idx]
        bias = np.full(lkv, NEG, np.float32)
        bias[: len(idx)] = 0.0
        in_maps.append(
            dict(
                shared,
                xT=np.ascontiguousarray(x[b].T).astype(BFNP),
                ctxT=np.ascontiguousarray(ctx_c.T).astype(BFNP),
                mask=np.ascontiguousarray(bias.reshape(kt, P).T),
            )
        )
    return lkv, in_maps


def kernel(x, context, context_mask, Wq, Wk, Wv, Wp, bp, q_norm_w, k_norm_w):
    global LAST_RESULTS
    x = np.asarray(x, dtype=np.float32)
    context = np.asarray(context, dtype=np.float32)
    context_mask = np.asarray(context_mask)
    lkv, in_maps = _prep_inputs(
        x, context, context_mask,
        np.asarray(Wq, np.float32), np.asarray(Wk, np.float32),
        np.asarray(Wv, np.float32), np.asarray(Wp, np.float32),
        np.asarray(bp, np.float32), np.asarray(q_norm_w, np.float32),
        np.asarray(k_norm_w, np.float32),
    )
    if lkv not in _CACHE:
        _CACHE[lkv] = _build(lkv)
    nc = _CACHE[lkv]
    res = bass_utils.run_bass_kernel_spmd(nc, in_maps, core_ids=list(range(B)))
    LAST_RESULTS = res
    return np.stack([res.results[b]["out"] for b in range(B)], axis=0)
